# revision 71
# baseline (speedup 1.0000x reference)
"""Multi-head causal attention (B=2, S=2048, D=1024, H=16) on 8 TRN2 NeuronCores.

Sharding: core c in 0..7 handles batch b = c // 4 and local head group
g = c % 4 (global heads 4g .. 4g+3).  Tensor-parallel over heads: each core
computes its heads' Q/K/V projections, causal attention, and a partial
output projection (W_out rows for its heads).  Host sums the 4 partials per
batch and adds b_out.

v2: bf16 storage everywhere (f32 PSUM accumulation), 128-granularity causal
trim, k-sliced startup DMA with k-outer chunk-0 projection, and phase
interleaving: qkv(c+1) / outproj(c-1) matmul blocks are emitted as filler
between attention ki-steps of chunk c so the tensor engine never waits on
the activation-engine exp cadence.
"""

from contextlib import ExitStack

import numpy as np
import ml_dtypes

import concourse.bass as bass
import concourse.mybir as mybir
import concourse.tile as tile
from concourse import bass_utils

F32 = mybir.dt.float32
BF16 = mybir.dt.bfloat16
EXP = mybir.ActivationFunctionType.Exp

B, S, D, H = 2, 2048, 1024, 16
HD = D // H          # 64
HL = 4               # heads per core
N_CORES = 8
SC = S // 512        # 4 q-chunks of 512
KT = S // 128        # 16 k-tiles of 128

_CACHE = {}

_NO_HOIST = {
    "AllEngineBarrier",
    "EventSemaphore",
    "UnconditionalBranch",
    "CompareAndBranch",
    "BranchHint",
    "IndirectBranch",
    "Halt",
    "Call",
    "OverlayCall",
    "NoOp",
}


def _fix_sync_waits(nc):
    """walrus codegen holds only one sync-wait per engine instruction; hoist
    excess waits onto same-engine NoOps inserted right before."""
    for fn in nc.m.functions:
        for blk in fn.blocks:
            insts = blk.instructions
            out = []
            changed = False
            for inst in insts:
                si = inst.sync_info
                if si is not None and inst.opcode not in _NO_HOIST:
                    waits = list(si.on_wait)
                    if len(waits) > 1:
                        for j, w in enumerate(waits[:-1]):
                            nop = mybir.InstNoOp(name=f"{inst.name}-wfix{j}")
                            nop.engine = inst.engine
                            nop.sync_info = mybir.SyncInfo(on_wait=[w], on_update=[])
                            out.append(nop)
                        inst.sync_info = mybir.SyncInfo(
                            on_wait=[waits[-1]], on_update=list(si.on_update)
                        )
                        changed = True
                out.append(inst)
            if changed:
                blk.instructions = out


def _build(reps=1, fix_waits=True, n_chunks=SC, trim=True, masks=True):
    nc = bass.Bass("TRN2", target_bir_lowering=False, debug=False,
                   num_devices=N_CORES)

    xT = nc.dram_tensor("xT", [128, 8, S], BF16, kind="ExternalInput").ap()
    w = nc.dram_tensor("w", [128, 8, 768], BF16, kind="ExternalInput").ap()
    wout = nc.dram_tensor("wout", [128, 2, D], BF16, kind="ExternalInput").ap()
    bq = nc.dram_tensor("bq", [128, 4], F32, kind="ExternalInput").ap()
    bv = nc.dram_tensor("bv", [128, 256], F32, kind="ExternalInput").ap()
    cmask = nc.dram_tensor("cmask", [128, 128], BF16, kind="ExternalInput").ap()
    outT = nc.dram_tensor("outT", [128, 8, S], BF16, kind="ExternalOutput").ap()

    with tile.TileContext(nc) as tc, ExitStack() as ctx:
        persist = ctx.enter_context(tc.tile_pool(name="persist", bufs=1))
        xpool = ctx.enter_context(tc.tile_pool(name="xp", bufs=3))
        epool = ctx.enter_context(tc.tile_pool(name="ep", bufs=3))
        rpool = ctx.enter_context(tc.tile_pool(name="rp", bufs=2))
        opool = ctx.enter_context(tc.tile_pool(name="op", bufs=4))
        # PSUM: scores 2x[128,1024] (4 banks) + AV accum 2x[128,512] (2) +
        # shared qkv/outproj/rb pool 2x[128,512] (2) = 8 banks
        ps_sc = ctx.enter_context(tc.tile_pool(name="ps_sc", bufs=2, space="PSUM"))
        ps_av = ctx.enter_context(tc.tile_pool(name="ps_av", bufs=2, space="PSUM"))
        ps_mm = ctx.enter_context(tc.tile_pool(name="ps_mm", bufs=2, space="PSUM"))

        w_sb = persist.tile([128, 8, 768], BF16, tag="w")
        wout_sb = persist.tile([128, 2, D], BF16, tag="wout")
        bq_sb = persist.tile([128, 4], F32, tag="bq")
        bv_sb = persist.tile([128, 256], F32, tag="bv")
        sel_sb = persist.tile([128, 256], BF16, tag="sel")
        cmask_sb = persist.tile([128, 128], BF16, tag="cmask")
        qT = persist.tile([128, 2, S], BF16, tag="qT")
        kT = persist.tile([128, 2, S], BF16, tag="kT")
        vn = persist.tile([128, KT, HL, 97], BF16, tag="vn")
        vraw = persist.tile([128, 2, S], BF16, tag="vraw")
        vnorm = persist.tile([128, 2, S], BF16, tag="vnorm")

        # device-built constants: the softmax-denominator ones column of the
        # augmented V — col 64 for even heads (den -> po row 64), col 96 for
        # odd heads (den -> po row 96, a valid mod-32 partition base) — and
        # the K=33 reciprocal-broadcast selector (rows 65..95 all zero).
        for h in range(HL):
            if h % 2 == 0:
                nc.vector.memset(vn[:, :, h, 64:65], 1.0)
            else:
                nc.vector.memset(vn[:, :, h, 64:96], 0.0)
                nc.vector.memset(vn[:, :, h, 96:97], 1.0)
        nc.vector.memset(sel_sb[64:96, 0:128], 0.0)
        nc.vector.memset(sel_sb[64:65, 0:64], 1.0)
        nc.vector.memset(sel_sb[96:97, 0:64], 0.0)
        nc.vector.memset(sel_sb[96:97, 64:128], 1.0)
        recrs = [persist.tile([128, 512], BF16, tag=f"recr{i}", name=f"recr{i}")
                 for i in range(2)]
        for r in recrs:
            nc.vector.memset(r[64:96, :], 0.0)

        # ---- startup DMA, k-sliced so the first matmuls start early ----
        xc0 = xpool.tile([128, 8, 512], BF16, tag="xc", name="xc0")
        nc.sync.dma_start(xc0[:, 0, :], xT[:, 0, 0:512])
        nc.scalar.dma_start(w_sb[:, 0, 0:512], w[:, 0, 0:512])
        nc.sync.dma_start(xc0[:, 1, :], xT[:, 1, 0:512])
        nc.scalar.dma_start(w_sb[:, 0, 512:768], w[:, 0, 512:768])
        nc.scalar.dma_start(w_sb[:, 1, :], w[:, 1, :])
        for k2 in range(1, 4):
            ks2 = slice(2 * k2, 2 * k2 + 2)
            nc.sync.dma_start(xc0[:, ks2, :], xT[:, ks2, 0:512])
            nc.scalar.dma_start(w_sb[:, ks2, :], w[:, ks2, :])
        nc.scalar.dma_start(bq_sb[:], bq)
        nc.scalar.dma_start(bv_sb[:], bv)
        nc.scalar.dma_start(cmask_sb[:], cmask)
        nc.scalar.dma_start(wout_sb[:], wout)

        # ---- chunk-0 qkv projection, k-outer (consumes slices as they land)
        # spA: [Q hp0 | K hp0], spB: [Q hp1 | K hp1]; pvA: [j0|j1], pvB: [j2|j3]
        # (hardware: at most ONE open matmul accumulation group per PSUM bank
        # — interleaved groups in a shared bank silently corrupt, so the four
        # qk accumulators get a bank each and V runs as sequential j-blocks)
        spA = ps_sc.tile([128, 1024], F32, tag="s", name="spA")
        spB = ps_sc.tile([128, 1024], F32, tag="s", name="spB")
        for k in range(8):
            st = k == 0
            sp_ = k == 7
            nc.tensor.matmul(spA[:, 0:512], w_sb[:, k, 0:128], xc0[:, k, :],
                             start=st, stop=sp_)
            nc.tensor.matmul(spA[:, 512:1024], w_sb[:, k, 256:384], xc0[:, k, :],
                             start=st, stop=sp_)
            nc.tensor.matmul(spB[:, 0:512], w_sb[:, k, 128:256], xc0[:, k, :],
                             start=st, stop=sp_)
            nc.tensor.matmul(spB[:, 512:1024], w_sb[:, k, 384:512], xc0[:, k, :],
                             start=st, stop=sp_)
        nc.vector.tensor_scalar_add(qT[:, 0, 0:512], spA[:, 0:512], bq_sb[:, 0:1])
        nc.vector.tensor_scalar_add(kT[:, 0, 0:512], spA[:, 512:1024], bq_sb[:, 2:3])
        nc.vector.tensor_scalar_add(qT[:, 1, 0:512], spB[:, 0:512], bq_sb[:, 1:2])
        nc.vector.tensor_scalar_add(kT[:, 1, 0:512], spB[:, 512:1024], bq_sb[:, 3:4])
        for st4 in range(4):
            pv = ps_av.tile([128, 512], F32, tag="av", name=f"pv0_{st4}")
            for k in range(8):
                nc.tensor.matmul(pv[:, 0:256],
                                 xc0[:, k, 128 * st4:128 * (st4 + 1)],
                                 w_sb[:, k, 512:768], start=(k == 0), stop=(k == 7))
            nc.vector.tensor_add(
                vn[:, st4, :, 0:64],
                pv[:, 0:256].rearrange("p (h d) -> p h d", h=4),
                bv_sb[:].rearrange("p (h d) -> p h d", h=4))

        xcs = {0: xc0}

        # ---- filler blocks: qkv projection of a later chunk / output
        # projection of an earlier chunk, emitted between attention steps ----
        def qk_halves(cn, m):
            # m: 0=Q hp0, 1=Q hp1, 2=K hp0, 3=K hp1 (matches w col + bias col)
            # split into two 4-k-step closures for finer filler granularity
            hold = {}

            def emit_a():
                hold["pm"] = ps_mm.tile([128, 512], F32, tag="mm",
                                        name=f"qk{cn}_{m}")
                for k in range(4):
                    nc.tensor.matmul(hold["pm"][:], w_sb[:, k, 128 * m:128 * (m + 1)],
                                     xcs[cn][:, k, :], start=(k == 0), stop=False)

            def emit_b():
                qs = slice(cn * 512, (cn + 1) * 512)
                pm = hold["pm"]
                for k in range(4, 8):
                    nc.tensor.matmul(pm[:], w_sb[:, k, 128 * m:128 * (m + 1)],
                                     xcs[cn][:, k, :], start=False, stop=(k == 7))
                dst = qT[:, m, qs] if m < 2 else kT[:, m - 2, qs]
                nc.vector.tensor_scalar_add(dst, pm[:], bq_sb[:, m:m + 1])
            return [emit_a, emit_b]

        def v_block(cn, j):
            def emit():
                pv = ps_mm.tile([128, 512], F32, tag="mm", name=f"v{cn}_{j}")
                for k in range(8):
                    nc.tensor.matmul(pv[:, 0:256],
                                     xcs[cn][:, k, 128 * j:128 * (j + 1)],
                                     w_sb[:, k, 512:768], start=(k == 0), stop=(k == 7))
                st4 = 4 * cn + j
                nc.vector.tensor_add(
                    vn[:, st4, :, 0:64],
                    pv[:, 0:256].rearrange("p (h d) -> p h d", h=4),
                    bv_sb[:].rearrange("p (h d) -> p h d", h=4))
            return emit

        ou_hold = {}
        pu_hold = {}

        def outproj_block(cn, m, copy_eng="dve", psum="mm"):
            # even m allocates a 2-block staging tile; odd m completes it and
            # issues one paired DMA (halves the per-transfer HWDGE overhead).
            # psum="sc": after the last scores, the 4 score banks are free —
            # pair two blocks per [128,1024] tile for deeper PU buffering.
            def emit():
                qs = slice(cn * 512, (cn + 1) * 512)
                if psum == "mm":
                    pu = ps_mm.tile([128, 512], F32, tag="mm",
                                    name=f"pu{cn}_{m}")[:]
                else:
                    if m % 2 == 0:
                        pu_hold[cn] = ps_sc.tile([128, 1024], F32, tag="s",
                                                 name=f"pu2{cn}_{m}")
                    pu = pu_hold[cn][:, 512 * (m % 2):512 * (m % 2) + 512]
                for t in range(2):
                    nc.tensor.matmul(pu, wout_sb[:, t, 128 * m:128 * (m + 1)],
                                     vnorm[:, t, qs], start=(t == 0), stop=(t == 1))
                if m % 2 == 0:
                    ou_hold[cn] = opool.tile([128, 2, 512], BF16, tag="ou",
                                             name=f"ou{cn}_{m}")
                ou = ou_hold[cn]
                dst = ou[:, m % 2, :]
                if copy_eng == "dve":
                    nc.vector.tensor_copy(dst, pu)
                elif copy_eng == "act":
                    nc.scalar.copy(dst, pu)
                if m % 2 == 1:
                    nc.sync.dma_start(outT[:, m - 1:m + 1, qs], ou[:])
            return emit

        # pending normalization chain of the previous head pair — emitted
        # right AFTER the next head pair's first exp is queued, so the Act
        # engine starts the next exp before the drain copies, and the PE has
        # scores/filler work while the reciprocal chain flows (crosses chunk
        # boundaries too)
        pending = {"norm": None}

        # ---- attention for one q-chunk, with filler drained between steps
        def attn_qc(qc, fillers, reserve=()):
            n_ki = 4 * qc + 4
            nsteps = 2 * n_ki + 2
            state = {"step": 0, "drained": 0}

            def tick(n=None):
                state["step"] += 1
                if n is None:
                    # at least one filler by step 1: right after a head-pair
                    # boundary the PE otherwise idles on the exp/recip chains
                    target = max(len(fillers) * state["step"] // nsteps,
                                 min(2, state["step"]))
                else:
                    target = state["drained"] + n
                while state["drained"] < min(target, len(fillers)):
                    fillers[state["drained"]]()
                    state["drained"] += 1

            qs = slice(qc * 512, (qc + 1) * 512)
            for hp in range(2):
                # po tiles are allocated lazily at the first AV so the pool
                # WAR lands after the previous pair's (deferred) drain copies
                po = [None, None]
                recr = recrs[(2 * qc + hp) % 2]
                es_hold = [None] * n_ki

                def do_av(ki, qc=qc, hp=hp, po=po, n_ki=n_ki, es_hold=es_hold):
                    if po[0] is None:
                        po[0] = ps_av.tile([128, 512], F32, tag="av",
                                           name=f"po{qc}{hp}0")
                        po[1] = ps_av.tile([128, 512], F32, tag="av",
                                           name=f"po{qc}{hp}1")
                    e, o, wdt = es_hold[ki]
                    for i in range(2):
                        h = 2 * hp + i
                        # V_aug.T @ E: rows 0..63 values, row 64 (even) or 96
                        # (odd) the softmax denominator (ones column of V_aug)
                        nc.tensor.matmul(
                            po[i][0:65 + 32 * i, o:512], vn[:, ki, h, 0:65 + 32 * i],
                            e[:, i * 512:i * 512 + wdt],
                            start=(ki == 0), stop=(ki == n_ki - 1),
                            skip_group_check=True)

                for ki in range(n_ki):
                    j = ki - 4 * qc
                    o = 128 * j if (trim and j >= 0) else 0
                    wdt = 512 - o
                    ks = slice(ki * 128, (ki + 1) * 128)
                    qsub = slice(qc * 512 + o, (qc + 1) * 512)
                    # head slabs live at bank-aligned offsets i*512 — the two
                    # tile_position score groups must not share a PSUM bank
                    sp = ps_sc.tile([128, 1024], F32, tag="s",
                                    name=f"sp{qc}{hp}{ki}")
                    for i in range(2):   # head within pair (row-packed)
                        vp = 64 * i
                        nc.tensor.matmul(
                            sp[:, i * 512:i * 512 + wdt],
                            kT[vp:vp + 64, hp, ks], qT[vp:vp + 64, hp, qsub],
                            start=True, stop=True, tile_position=(vp, 0))
                    e = epool.tile([128, 1024], BF16, tag="e",
                                   name=f"e{qc}{hp}{ki}")
                    if wdt == 512:
                        nc.scalar.activation(e[:], sp[:], EXP, scale=0.125)
                    else:
                        sp3 = sp[:].rearrange("p (t q) -> p t q", t=2)
                        e3 = e[:].rearrange("p (t q) -> p t q", t=2)
                        nc.scalar.activation(e3[:, :, 0:wdt], sp3[:, :, 0:wdt],
                                             EXP, scale=0.125)
                    if masks and j >= 0:
                        # diagonal tile: with o=128j the invalid region is
                        # always the leading 128-col triangle (kk > qq)
                        mw = min(128, wdt)
                        for i in range(2):
                            es = e[:, i * 512:i * 512 + mw]
                            nc.vector.tensor_mul(es, es, cmask_sb[:, 0:mw])
                    es_hold[ki] = (e, o, wdt)
                    if ki == 0 and pending["norm"] is not None:
                        pending["norm"]()
                        pending["norm"] = None
                    if ki >= 1:
                        do_av(ki - 1)
                    tick()
                do_av(n_ki - 1)
                tick()

                # normalization: per-head reciprocal of the denominator rows,
                # broadcast onto value partitions via one K=33 selector matmul.
                # Denominators: even head on po[0] row 64, odd head on po[1]
                # row 96 (the odd V_aug ones column sits at col 96), so both
                # reciprocals stay partition-aligned (rows 65..95 are zero).
                def norm(qc=qc, hp=hp, po=po, recr=recr, qs=qs,
                         res=tuple(reserve) if (hp == 1 and reserve) else ()):
                    stage = rpool.tile([128, 512], BF16, tag="stage",
                                       name=f"st{qc}{hp}")
                    nc.scalar.copy(stage[0:64, :], po[1][0:64, :])
                    # sync ring: out-DMAs queued behind this were emitted in
                    # the same window, so the short stage-copy wait cannot
                    # head-of-line-block them for long
                    nc.sync.dma_start(vraw[64:128, hp, qs], stage[0:64, :])
                    with nc.allow_low_precision(reason="bf16 softmax recip"):
                        nc.vector.reciprocal(recr[96:97, :], po[1][96:97, :])
                        nc.vector.reciprocal(recr[64:65, :], po[0][64:65, :])
                    nc.scalar.copy(vraw[0:64, hp, qs], po[0][0:64, :])
                    if res:
                        res[0]()
                        res[1]()
                    else:
                        tick(n=2)
                    rb = ps_mm.tile([128, 512], F32, tag="mm", name=f"rb{qc}{hp}")
                    nc.tensor.matmul(rb[:], sel_sb[64:97, 0:128], recr[64:97, :],
                                     start=True, stop=True, tile_position=(64, 0))
                    for r in res[2:]:
                        r()
                    nc.vector.tensor_mul(vnorm[:, hp, qs], vraw[:, hp, qs], rb[:])
                pending["norm"] = norm
            # any fillers not yet drained
            while state["drained"] < len(fillers):
                fillers[state["drained"]]()
                state["drained"] += 1

        # ---- main schedule: attn(c) with qkv(c+1) as filler; all deferrable
        # output projections (chunks 0..2) land in attn(3), whose exp cadence
        # otherwise starves the tensor engine; outproj(3) is the tail.  In
        # attn(3) the copies ride the idle Pool engine so the DVE recip ->
        # vnorm chain stays short; the last few blocks are reserved to keep
        # the PE warm through the final normalization chain.
        for c in range(n_chunks):
            fillers, reserve = [], []
            if c + 1 < n_chunks:
                xc = xpool.tile([128, 8, 512], BF16, tag="xc", name=f"xc{c+1}")
                qsn = slice((c + 1) * 512, (c + 2) * 512)
                nc.sync.dma_start(xc[:, 0:4, :], xT[:, 0:4, qsn])
                nc.sync.dma_start(xc[:, 4:8, :], xT[:, 4:8, qsn])
                xcs[c + 1] = xc
                for m in range(4):
                    fillers += qk_halves(c + 1, m)
                fillers += [v_block(c + 1, j) for j in range(4)]
            else:
                # GPSIMD cannot touch PSUM, so drain copies ride DVE (fillers)
                # and Act (reserve, where the exp pipeline has already drained)
                for cn in range(n_chunks - 2):
                    fillers += [outproj_block(cn, m, "dve") for m in range(8)]
                fillers += [outproj_block(n_chunks - 2, m, "dve") for m in range(4)]
                reserve = [outproj_block(n_chunks - 2, m, "act") for m in range(4, 8)]
            attn_qc(c, fillers, reserve)
        # the last head pair's normalization, then the tail output projection
        pending["norm"]()
        pending["norm"] = None
        for m in range(8):
            outproj_block(n_chunks - 1, m, copy_eng=("dve" if m % 2 else "act"))()

    if fix_waits:
        _fix_sync_waits(nc)
    return nc


def _get_nc():
    if "nc" not in _CACHE:
        _CACHE["nc"] = _build()
    return _CACHE["nc"]


def _make_cmask() -> np.ndarray:
    """cmask[128, 128]: c[kk, qq] = 1 iff kk <= qq (relative causal triangle
    applied to the leading 128 cols of every diagonal score tile)."""
    kk = np.arange(128)[:, None]
    qq = np.arange(128)[None, :]
    return (kk <= qq).astype(np.float32)


def kernel(x, W_qkv, b_qkv, W_out, b_out):
    x = np.asarray(x, np.float32)
    W_qkv = np.asarray(W_qkv, np.float32)
    b_qkv = np.asarray(b_qkv, np.float32)
    W_out = np.asarray(W_out, np.float32)
    b_out = np.asarray(b_out, np.float32)

    nc = _get_nc()
    cmask = _make_cmask().astype(ml_dtypes.bfloat16)

    in_maps = []
    for c in range(N_CORES):
        b, g = divmod(c, 4)
        heads = [4 * g + i for i in range(HL)]
        # reorder W_qkv columns: [Q(h0..h3) | K(h0..h3) | V(h0..h3)]
        qcols = np.concatenate([W_qkv[:, h * 192:h * 192 + 64] for h in heads], 1)
        kcols = np.concatenate([W_qkv[:, h * 192 + 64:h * 192 + 128] for h in heads], 1)
        vcols = np.concatenate([W_qkv[:, h * 192 + 128:h * 192 + 192] for h in heads], 1)
        wsh = np.concatenate([qcols, kcols, vcols], 1)          # [1024, 768]
        bqv = np.concatenate([b_qkv[h * 192:h * 192 + 64] for h in heads])
        bkv = np.concatenate([b_qkv[h * 192 + 64:h * 192 + 128] for h in heads])
        bvv = np.concatenate([b_qkv[h * 192 + 128:h * 192 + 192] for h in heads])
        wo = W_out[g * 256:(g + 1) * 256, :]                    # [256, 1024]

        xT = x[b].T.reshape(8, 128, S).transpose(1, 0, 2)       # [128, 8, S]
        wsh3 = wsh.reshape(8, 128, 768).transpose(1, 0, 2)      # [128, 8, 768]
        wo3 = wo.reshape(2, 128, D).transpose(1, 0, 2)          # [128, 2, D]
        bq2 = np.concatenate([bqv, bkv]).reshape(4, 128).T      # [128, 4]
        bv2 = np.broadcast_to(bvv, (128, 256))                  # [128, 256]

        in_maps.append({
            "xT": np.ascontiguousarray(xT).astype(ml_dtypes.bfloat16),
            "w": np.ascontiguousarray(wsh3).astype(ml_dtypes.bfloat16),
            "wout": np.ascontiguousarray(wo3).astype(ml_dtypes.bfloat16),
            "bq": np.ascontiguousarray(bq2),
            "bv": np.ascontiguousarray(bv2),
            "cmask": np.ascontiguousarray(cmask),
        })

    _CACHE["in_maps"] = in_maps
    res = bass_utils.run_bass_kernel_spmd(nc, in_maps, core_ids=list(range(N_CORES)))

    out = np.zeros((B, S, D), np.float32)
    for c in range(N_CORES):
        b = c // 4
        oT = np.asarray(res.results[c]["outT"]).astype(np.float32)  # [128, 8, S]
        out[b] += oT.transpose(1, 0, 2).reshape(D, S).T
    out += b_out
    return out


# revision 77
# speedup vs baseline: 1.0020x; 1.0020x over previous
"""Multi-head causal attention (B=2, S=2048, D=1024, H=16) on 8 TRN2 NeuronCores.

Sharding: core c in 0..7 handles batch b = c // 4 and local head group
g = c % 4 (global heads 4g .. 4g+3).  Tensor-parallel over heads: each core
computes its heads' Q/K/V projections, causal attention, and a partial
output projection (W_out rows for its heads).  Host sums the 4 partials per
batch and adds b_out.

v2: bf16 storage everywhere (f32 PSUM accumulation), 128-granularity causal
trim, k-sliced startup DMA with k-outer chunk-0 projection, and phase
interleaving: qkv(c+1) / outproj(c-1) matmul blocks are emitted as filler
between attention ki-steps of chunk c so the tensor engine never waits on
the activation-engine exp cadence.
"""

from contextlib import ExitStack

import numpy as np
import ml_dtypes

import concourse.bass as bass
import concourse.mybir as mybir
import concourse.tile as tile
from concourse import bass_utils

F32 = mybir.dt.float32
BF16 = mybir.dt.bfloat16
EXP = mybir.ActivationFunctionType.Exp

B, S, D, H = 2, 2048, 1024, 16
HD = D // H          # 64
HL = 4               # heads per core
N_CORES = 8
SC = S // 512        # 4 q-chunks of 512
KT = S // 128        # 16 k-tiles of 128

_CACHE = {}

_NO_HOIST = {
    "AllEngineBarrier",
    "EventSemaphore",
    "UnconditionalBranch",
    "CompareAndBranch",
    "BranchHint",
    "IndirectBranch",
    "Halt",
    "Call",
    "OverlayCall",
    "NoOp",
}


def _fix_sync_waits(nc):
    """walrus codegen holds only one sync-wait per engine instruction; hoist
    excess waits onto same-engine NoOps inserted right before."""
    for fn in nc.m.functions:
        for blk in fn.blocks:
            insts = blk.instructions
            out = []
            changed = False
            for inst in insts:
                si = inst.sync_info
                if si is not None and inst.opcode not in _NO_HOIST:
                    waits = list(si.on_wait)
                    if len(waits) > 1:
                        for j, w in enumerate(waits[:-1]):
                            nop = mybir.InstNoOp(name=f"{inst.name}-wfix{j}")
                            nop.engine = inst.engine
                            nop.sync_info = mybir.SyncInfo(on_wait=[w], on_update=[])
                            out.append(nop)
                        inst.sync_info = mybir.SyncInfo(
                            on_wait=[waits[-1]], on_update=list(si.on_update)
                        )
                        changed = True
                out.append(inst)
            if changed:
                blk.instructions = out


def _build(reps=1, fix_waits=True, n_chunks=SC, trim=True, masks=True):
    nc = bass.Bass("TRN2", target_bir_lowering=False, debug=False,
                   num_devices=N_CORES)

    xT = nc.dram_tensor("xT", [128, 8, S], BF16, kind="ExternalInput").ap()
    w = nc.dram_tensor("w", [128, 8, 768], BF16, kind="ExternalInput").ap()
    wout = nc.dram_tensor("wout", [128, 2, D], BF16, kind="ExternalInput").ap()
    bq = nc.dram_tensor("bq", [128, 4], F32, kind="ExternalInput").ap()
    bv = nc.dram_tensor("bv", [128, 256], F32, kind="ExternalInput").ap()
    cmask = nc.dram_tensor("cmask", [128, 128], BF16, kind="ExternalInput").ap()
    outT = nc.dram_tensor("outT", [128, 8, S], BF16, kind="ExternalOutput").ap()

    with tile.TileContext(nc) as tc, ExitStack() as ctx:
        persist = ctx.enter_context(tc.tile_pool(name="persist", bufs=1))
        xpool = ctx.enter_context(tc.tile_pool(name="xp", bufs=3))
        epool = ctx.enter_context(tc.tile_pool(name="ep", bufs=3))
        rpool = ctx.enter_context(tc.tile_pool(name="rp", bufs=2))
        opool = ctx.enter_context(tc.tile_pool(name="op", bufs=4))
        # PSUM: scores 2x[128,1024] (4 banks) + AV accum 2x[128,512] (2) +
        # shared qkv/outproj/rb pool 2x[128,512] (2) = 8 banks
        ps_sc = ctx.enter_context(tc.tile_pool(name="ps_sc", bufs=2, space="PSUM"))
        ps_av = ctx.enter_context(tc.tile_pool(name="ps_av", bufs=2, space="PSUM"))
        ps_mm = ctx.enter_context(tc.tile_pool(name="ps_mm", bufs=2, space="PSUM"))

        w_sb = persist.tile([128, 8, 768], BF16, tag="w")
        wout_sb = persist.tile([128, 2, D], BF16, tag="wout")
        bq_sb = persist.tile([128, 4], F32, tag="bq")
        bv_sb = persist.tile([128, 256], F32, tag="bv")
        sel_sb = persist.tile([128, 256], BF16, tag="sel")
        cmask_sb = persist.tile([128, 128], BF16, tag="cmask")
        qT = persist.tile([128, 2, S], BF16, tag="qT")
        kT = persist.tile([128, 2, S], BF16, tag="kT")
        vn = persist.tile([128, KT, HL, 97], BF16, tag="vn")
        vraw = persist.tile([128, 2, S], BF16, tag="vraw")
        vnorm = persist.tile([128, 2, S], BF16, tag="vnorm")

        # device-built constants: the softmax-denominator ones column of the
        # augmented V — col 64 for even heads (den -> po row 64), col 96 for
        # odd heads (den -> po row 96, a valid mod-32 partition base) — and
        # the K=33 reciprocal-broadcast selector (rows 65..95 all zero).
        nc.vector.memset(sel_sb[64:96, 0:128], 0.0)
        nc.vector.memset(sel_sb[64:65, 0:64], 1.0)
        nc.vector.memset(sel_sb[96:97, 0:64], 0.0)
        nc.vector.memset(sel_sb[96:97, 64:128], 1.0)
        for h in range(HL):
            if h % 2 == 0:
                nc.vector.memset(vn[:, :, h, 64:65], 1.0)
            else:
                nc.vector.memset(vn[:, :, h, 64:96], 0.0)
                nc.vector.memset(vn[:, :, h, 96:97], 1.0)
        recrs = [persist.tile([128, 512], BF16, tag=f"recr{i}", name=f"recr{i}")
                 for i in range(2)]
        for r in recrs:
            nc.vector.memset(r[64:96, :], 0.0)

        # ---- startup DMA, k-sliced so the first matmuls start early ----
        xc0 = xpool.tile([128, 8, 512], BF16, tag="xc", name="xc0")
        nc.sync.dma_start(xc0[:, 0, :], xT[:, 0, 0:512])
        nc.scalar.dma_start(w_sb[:, 0, 0:512], w[:, 0, 0:512])
        nc.sync.dma_start(xc0[:, 1, :], xT[:, 1, 0:512])
        nc.scalar.dma_start(w_sb[:, 0, 512:768], w[:, 0, 512:768])
        nc.scalar.dma_start(w_sb[:, 1, :], w[:, 1, :])
        for k2 in range(1, 4):
            ks2 = slice(2 * k2, 2 * k2 + 2)
            nc.sync.dma_start(xc0[:, ks2, :], xT[:, ks2, 0:512])
            nc.scalar.dma_start(w_sb[:, ks2, :], w[:, ks2, :])
        nc.scalar.dma_start(bq_sb[:], bq)
        nc.scalar.dma_start(bv_sb[:], bv)
        nc.scalar.dma_start(cmask_sb[:], cmask)
        nc.scalar.dma_start(wout_sb[:], wout)

        # ---- PE clock warm-up: junk matmuls over the (memset) selector rows
        # while the first x/w slices stream in, so the p-state ramp completes
        # before the first real matmul (the result bank is never read)
        warm = ps_mm.tile([128, 512], F32, tag="mm", name="warm")
        for i in range(14):
            nc.tensor.matmul(warm[:, 0:256], sel_sb[64:96, 0:128],
                             sel_sb[64:96, 0:256], start=True, stop=True)

        # ---- chunk-0 qkv projection, k-outer (consumes slices as they land)
        # spA: [Q hp0 | K hp0], spB: [Q hp1 | K hp1]; pvA: [j0|j1], pvB: [j2|j3]
        # (hardware: at most ONE open matmul accumulation group per PSUM bank
        # — interleaved groups in a shared bank silently corrupt, so the four
        # qk accumulators get a bank each and V runs as sequential j-blocks)
        spA = ps_sc.tile([128, 1024], F32, tag="s", name="spA")
        spB = ps_sc.tile([128, 1024], F32, tag="s", name="spB")
        for k in range(8):
            st = k == 0
            sp_ = k == 7
            nc.tensor.matmul(spA[:, 0:512], w_sb[:, k, 0:128], xc0[:, k, :],
                             start=st, stop=sp_)
            nc.tensor.matmul(spA[:, 512:1024], w_sb[:, k, 256:384], xc0[:, k, :],
                             start=st, stop=sp_)
            nc.tensor.matmul(spB[:, 0:512], w_sb[:, k, 128:256], xc0[:, k, :],
                             start=st, stop=sp_)
            nc.tensor.matmul(spB[:, 512:1024], w_sb[:, k, 384:512], xc0[:, k, :],
                             start=st, stop=sp_)
        nc.vector.tensor_scalar_add(qT[:, 0, 0:512], spA[:, 0:512], bq_sb[:, 0:1])
        nc.vector.tensor_scalar_add(kT[:, 0, 0:512], spA[:, 512:1024], bq_sb[:, 2:3])
        nc.vector.tensor_scalar_add(qT[:, 1, 0:512], spB[:, 0:512], bq_sb[:, 1:2])
        nc.vector.tensor_scalar_add(kT[:, 1, 0:512], spB[:, 512:1024], bq_sb[:, 3:4])
        for st4 in range(4):
            pv = ps_av.tile([128, 512], F32, tag="av", name=f"pv0_{st4}")
            for k in range(8):
                nc.tensor.matmul(pv[:, 0:256],
                                 xc0[:, k, 128 * st4:128 * (st4 + 1)],
                                 w_sb[:, k, 512:768], start=(k == 0), stop=(k == 7))
            nc.vector.tensor_add(
                vn[:, st4, :, 0:64],
                pv[:, 0:256].rearrange("p (h d) -> p h d", h=4),
                bv_sb[:].rearrange("p (h d) -> p h d", h=4))

        xcs = {0: xc0}

        # ---- filler blocks: qkv projection of a later chunk / output
        # projection of an earlier chunk, emitted between attention steps ----
        def qk_halves(cn, m):
            # m: 0=Q hp0, 1=Q hp1, 2=K hp0, 3=K hp1 (matches w col + bias col)
            # split into two 4-k-step closures for finer filler granularity
            hold = {}

            def emit_a():
                hold["pm"] = ps_mm.tile([128, 512], F32, tag="mm",
                                        name=f"qk{cn}_{m}")
                for k in range(4):
                    nc.tensor.matmul(hold["pm"][:], w_sb[:, k, 128 * m:128 * (m + 1)],
                                     xcs[cn][:, k, :], start=(k == 0), stop=False)

            def emit_b():
                qs = slice(cn * 512, (cn + 1) * 512)
                pm = hold["pm"]
                for k in range(4, 8):
                    nc.tensor.matmul(pm[:], w_sb[:, k, 128 * m:128 * (m + 1)],
                                     xcs[cn][:, k, :], start=False, stop=(k == 7))
                dst = qT[:, m, qs] if m < 2 else kT[:, m - 2, qs]
                nc.vector.tensor_scalar_add(dst, pm[:], bq_sb[:, m:m + 1])
            return [emit_a, emit_b]

        def v_block(cn, j):
            def emit():
                pv = ps_mm.tile([128, 512], F32, tag="mm", name=f"v{cn}_{j}")
                for k in range(8):
                    nc.tensor.matmul(pv[:, 0:256],
                                     xcs[cn][:, k, 128 * j:128 * (j + 1)],
                                     w_sb[:, k, 512:768], start=(k == 0), stop=(k == 7))
                st4 = 4 * cn + j
                nc.vector.tensor_add(
                    vn[:, st4, :, 0:64],
                    pv[:, 0:256].rearrange("p (h d) -> p h d", h=4),
                    bv_sb[:].rearrange("p (h d) -> p h d", h=4))
            return emit

        ou_hold = {}
        pu_hold = {}

        def outproj_block(cn, m, copy_eng="dve", psum="mm"):
            # even m allocates a 2-block staging tile; odd m completes it and
            # issues one paired DMA (halves the per-transfer HWDGE overhead).
            # psum="sc": after the last scores, the 4 score banks are free —
            # pair two blocks per [128,1024] tile for deeper PU buffering.
            def emit():
                qs = slice(cn * 512, (cn + 1) * 512)
                if psum == "mm":
                    pu = ps_mm.tile([128, 512], F32, tag="mm",
                                    name=f"pu{cn}_{m}")[:]
                else:
                    if m % 2 == 0:
                        pu_hold[cn] = ps_sc.tile([128, 1024], F32, tag="s",
                                                 name=f"pu2{cn}_{m}")
                    pu = pu_hold[cn][:, 512 * (m % 2):512 * (m % 2) + 512]
                for t in range(2):
                    nc.tensor.matmul(pu, wout_sb[:, t, 128 * m:128 * (m + 1)],
                                     vnorm[:, t, qs], start=(t == 0), stop=(t == 1))
                if m % 2 == 0:
                    ou_hold[cn] = opool.tile([128, 2, 512], BF16, tag="ou",
                                             name=f"ou{cn}_{m}")
                ou = ou_hold[cn]
                dst = ou[:, m % 2, :]
                if copy_eng == "dve":
                    nc.vector.tensor_copy(dst, pu)
                elif copy_eng == "act":
                    nc.scalar.copy(dst, pu)
                if m % 2 == 1:
                    nc.sync.dma_start(outT[:, m - 1:m + 1, qs], ou[:])
            return emit

        # pending normalization chain of the previous head pair — emitted
        # right AFTER the next head pair's first exp is queued, so the Act
        # engine starts the next exp before the drain copies, and the PE has
        # scores/filler work while the reciprocal chain flows (crosses chunk
        # boundaries too)
        pending = {"norm": None}

        # ---- attention for one q-chunk, with filler drained between steps
        def attn_qc(qc, fillers, reserve=()):
            n_ki = 4 * qc + 4
            nsteps = 2 * n_ki + 2
            state = {"step": 0, "drained": 0}

            def tick(n=None):
                state["step"] += 1
                if n is None:
                    # at least one filler by step 1: right after a head-pair
                    # boundary the PE otherwise idles on the exp/recip chains
                    target = max(len(fillers) * state["step"] // nsteps,
                                 min(2, state["step"]))
                else:
                    target = state["drained"] + n
                while state["drained"] < min(target, len(fillers)):
                    fillers[state["drained"]]()
                    state["drained"] += 1

            qs = slice(qc * 512, (qc + 1) * 512)
            for hp in range(2):
                # po tiles are allocated lazily at the first AV so the pool
                # WAR lands after the previous pair's (deferred) drain copies
                po = [None, None]
                recr = recrs[(2 * qc + hp) % 2]
                es_hold = [None] * n_ki

                def do_av(ki, qc=qc, hp=hp, po=po, n_ki=n_ki, es_hold=es_hold):
                    if po[0] is None:
                        po[0] = ps_av.tile([128, 512], F32, tag="av",
                                           name=f"po{qc}{hp}0")
                        po[1] = ps_av.tile([128, 512], F32, tag="av",
                                           name=f"po{qc}{hp}1")
                    e, o, wdt = es_hold[ki]
                    for i in range(2):
                        h = 2 * hp + i
                        # V_aug.T @ E: rows 0..63 values, row 64 (even) or 96
                        # (odd) the softmax denominator (ones column of V_aug)
                        nc.tensor.matmul(
                            po[i][0:65 + 32 * i, o:512], vn[:, ki, h, 0:65 + 32 * i],
                            e[:, i * 512:i * 512 + wdt],
                            start=(ki == 0), stop=(ki == n_ki - 1),
                            skip_group_check=True)

                for ki in range(n_ki):
                    j = ki - 4 * qc
                    o = 128 * j if (trim and j >= 0) else 0
                    wdt = 512 - o
                    ks = slice(ki * 128, (ki + 1) * 128)
                    qsub = slice(qc * 512 + o, (qc + 1) * 512)
                    # head slabs live at bank-aligned offsets i*512 — the two
                    # tile_position score groups must not share a PSUM bank
                    sp = ps_sc.tile([128, 1024], F32, tag="s",
                                    name=f"sp{qc}{hp}{ki}")
                    for i in range(2):   # head within pair (row-packed)
                        vp = 64 * i
                        nc.tensor.matmul(
                            sp[:, i * 512:i * 512 + wdt],
                            kT[vp:vp + 64, hp, ks], qT[vp:vp + 64, hp, qsub],
                            start=True, stop=True, tile_position=(vp, 0))
                    e = epool.tile([128, 1024], BF16, tag="e",
                                   name=f"e{qc}{hp}{ki}")
                    if wdt == 512:
                        nc.scalar.activation(e[:], sp[:], EXP, scale=0.125)
                    else:
                        sp3 = sp[:].rearrange("p (t q) -> p t q", t=2)
                        e3 = e[:].rearrange("p (t q) -> p t q", t=2)
                        nc.scalar.activation(e3[:, :, 0:wdt], sp3[:, :, 0:wdt],
                                             EXP, scale=0.125)
                    if masks and j >= 0:
                        # diagonal tile: with o=128j the invalid region is
                        # always the leading 128-col triangle (kk > qq)
                        mw = min(128, wdt)
                        for i in range(2):
                            es = e[:, i * 512:i * 512 + mw]
                            nc.vector.tensor_mul(es, es, cmask_sb[:, 0:mw])
                    es_hold[ki] = (e, o, wdt)
                    if ki == 0 and pending["norm"] is not None:
                        pending["norm"]()
                        pending["norm"] = None
                    if ki >= 1:
                        do_av(ki - 1)
                    tick()
                do_av(n_ki - 1)
                tick()

                # normalization: per-head reciprocal of the denominator rows,
                # broadcast onto value partitions via one K=33 selector matmul.
                # Denominators: even head on po[0] row 64, odd head on po[1]
                # row 96 (the odd V_aug ones column sits at col 96), so both
                # reciprocals stay partition-aligned (rows 65..95 are zero).
                def norm(qc=qc, hp=hp, po=po, recr=recr, qs=qs,
                         res=tuple(reserve) if (hp == 1 and reserve) else ()):
                    stage = rpool.tile([128, 512], BF16, tag="stage",
                                       name=f"st{qc}{hp}")
                    nc.scalar.copy(stage[0:64, :], po[1][0:64, :])
                    # sync ring: out-DMAs queued behind this were emitted in
                    # the same window, so the short stage-copy wait cannot
                    # head-of-line-block them for long
                    nc.sync.dma_start(vraw[64:128, hp, qs], stage[0:64, :])
                    with nc.allow_low_precision(reason="bf16 softmax recip"):
                        nc.vector.reciprocal(recr[96:97, :], po[1][96:97, :])
                        nc.vector.reciprocal(recr[64:65, :], po[0][64:65, :])
                    nc.scalar.copy(vraw[0:64, hp, qs], po[0][0:64, :])
                    if res:
                        res[0]()
                        res[1]()
                    else:
                        tick(n=2)
                    rb = ps_mm.tile([128, 512], F32, tag="mm", name=f"rb{qc}{hp}")
                    nc.tensor.matmul(rb[:], sel_sb[64:97, 0:128], recr[64:97, :],
                                     start=True, stop=True, tile_position=(64, 0))
                    for r in res[2:]:
                        r()
                    nc.vector.tensor_mul(vnorm[:, hp, qs], vraw[:, hp, qs], rb[:])
                pending["norm"] = norm
            # any fillers not yet drained
            while state["drained"] < len(fillers):
                fillers[state["drained"]]()
                state["drained"] += 1

        # ---- main schedule: attn(c) with qkv(c+1) as filler; all deferrable
        # output projections (chunks 0..2) land in attn(3), whose exp cadence
        # otherwise starves the tensor engine; outproj(3) is the tail.  In
        # attn(3) the copies ride the idle Pool engine so the DVE recip ->
        # vnorm chain stays short; the last few blocks are reserved to keep
        # the PE warm through the final normalization chain.
        for c in range(n_chunks):
            fillers, reserve = [], []
            if c + 1 < n_chunks:
                xc = xpool.tile([128, 8, 512], BF16, tag="xc", name=f"xc{c+1}")
                qsn = slice((c + 1) * 512, (c + 2) * 512)
                nc.sync.dma_start(xc[:, 0:4, :], xT[:, 0:4, qsn])
                nc.sync.dma_start(xc[:, 4:8, :], xT[:, 4:8, qsn])
                xcs[c + 1] = xc
                for m in range(4):
                    fillers += qk_halves(c + 1, m)
                fillers += [v_block(c + 1, j) for j in range(4)]
            else:
                # GPSIMD cannot touch PSUM, so drain copies ride DVE (fillers)
                # and Act (reserve, where the exp pipeline has already drained)
                for cn in range(n_chunks - 2):
                    fillers += [outproj_block(cn, m, "dve") for m in range(8)]
                fillers += [outproj_block(n_chunks - 2, m, "dve") for m in range(4)]
                reserve = [outproj_block(n_chunks - 2, m, "act") for m in range(4, 8)]
            attn_qc(c, fillers, reserve)
        # the last head pair's normalization, then the tail output projection
        pending["norm"]()
        pending["norm"] = None
        for m in range(8):
            outproj_block(n_chunks - 1, m, copy_eng=("dve" if m % 2 else "act"))()

    if fix_waits:
        _fix_sync_waits(nc)
    return nc


def _get_nc():
    if "nc" not in _CACHE:
        _CACHE["nc"] = _build()
    return _CACHE["nc"]


def _make_cmask() -> np.ndarray:
    """cmask[128, 128]: c[kk, qq] = 1 iff kk <= qq (relative causal triangle
    applied to the leading 128 cols of every diagonal score tile)."""
    kk = np.arange(128)[:, None]
    qq = np.arange(128)[None, :]
    return (kk <= qq).astype(np.float32)


def kernel(x, W_qkv, b_qkv, W_out, b_out):
    x = np.asarray(x, np.float32)
    W_qkv = np.asarray(W_qkv, np.float32)
    b_qkv = np.asarray(b_qkv, np.float32)
    W_out = np.asarray(W_out, np.float32)
    b_out = np.asarray(b_out, np.float32)

    nc = _get_nc()
    cmask = _make_cmask().astype(ml_dtypes.bfloat16)

    in_maps = []
    for c in range(N_CORES):
        b, g = divmod(c, 4)
        heads = [4 * g + i for i in range(HL)]
        # reorder W_qkv columns: [Q(h0..h3) | K(h0..h3) | V(h0..h3)]
        qcols = np.concatenate([W_qkv[:, h * 192:h * 192 + 64] for h in heads], 1)
        kcols = np.concatenate([W_qkv[:, h * 192 + 64:h * 192 + 128] for h in heads], 1)
        vcols = np.concatenate([W_qkv[:, h * 192 + 128:h * 192 + 192] for h in heads], 1)
        wsh = np.concatenate([qcols, kcols, vcols], 1)          # [1024, 768]
        bqv = np.concatenate([b_qkv[h * 192:h * 192 + 64] for h in heads])
        bkv = np.concatenate([b_qkv[h * 192 + 64:h * 192 + 128] for h in heads])
        bvv = np.concatenate([b_qkv[h * 192 + 128:h * 192 + 192] for h in heads])
        wo = W_out[g * 256:(g + 1) * 256, :]                    # [256, 1024]

        xT = x[b].T.reshape(8, 128, S).transpose(1, 0, 2)       # [128, 8, S]
        wsh3 = wsh.reshape(8, 128, 768).transpose(1, 0, 2)      # [128, 8, 768]
        wo3 = wo.reshape(2, 128, D).transpose(1, 0, 2)          # [128, 2, D]
        bq2 = np.concatenate([bqv, bkv]).reshape(4, 128).T      # [128, 4]
        bv2 = np.broadcast_to(bvv, (128, 256))                  # [128, 256]

        in_maps.append({
            "xT": np.ascontiguousarray(xT).astype(ml_dtypes.bfloat16),
            "w": np.ascontiguousarray(wsh3).astype(ml_dtypes.bfloat16),
            "wout": np.ascontiguousarray(wo3).astype(ml_dtypes.bfloat16),
            "bq": np.ascontiguousarray(bq2),
            "bv": np.ascontiguousarray(bv2),
            "cmask": np.ascontiguousarray(cmask),
        })

    _CACHE["in_maps"] = in_maps
    res = bass_utils.run_bass_kernel_spmd(nc, in_maps, core_ids=list(range(N_CORES)))

    out = np.zeros((B, S, D), np.float32)
    for c in range(N_CORES):
        b = c // 4
        oT = np.asarray(res.results[c]["outT"]).astype(np.float32)  # [128, 8, S]
        out[b] += oT.transpose(1, 0, 2).reshape(D, S).T
    out += b_out
    return out


# revision 83
# speedup vs baseline: 1.0033x; 1.0013x over previous
"""Multi-head causal attention (B=2, S=2048, D=1024, H=16) on 8 TRN2 NeuronCores.

Sharding: core c in 0..7 handles batch b = c // 4 and local head group
g = c % 4 (global heads 4g .. 4g+3).  Tensor-parallel over heads: each core
computes its heads' Q/K/V projections, causal attention, and a partial
output projection (W_out rows for its heads).  Host sums the 4 partials per
batch and adds b_out.

v2: bf16 storage everywhere (f32 PSUM accumulation), 128-granularity causal
trim, k-sliced startup DMA with k-outer chunk-0 projection, and phase
interleaving: qkv(c+1) / outproj(c-1) matmul blocks are emitted as filler
between attention ki-steps of chunk c so the tensor engine never waits on
the activation-engine exp cadence.
"""

from contextlib import ExitStack

import numpy as np
import ml_dtypes

import concourse.bass as bass
import concourse.mybir as mybir
import concourse.tile as tile
from concourse import bass_utils

F32 = mybir.dt.float32
BF16 = mybir.dt.bfloat16
EXP = mybir.ActivationFunctionType.Exp

B, S, D, H = 2, 2048, 1024, 16
HD = D // H          # 64
HL = 4               # heads per core
N_CORES = 8
SC = S // 512        # 4 q-chunks of 512
KT = S // 128        # 16 k-tiles of 128

_CACHE = {}

_NO_HOIST = {
    "AllEngineBarrier",
    "EventSemaphore",
    "UnconditionalBranch",
    "CompareAndBranch",
    "BranchHint",
    "IndirectBranch",
    "Halt",
    "Call",
    "OverlayCall",
    "NoOp",
}


def _fix_sync_waits(nc):
    """walrus codegen holds only one sync-wait per engine instruction; hoist
    excess waits onto same-engine NoOps inserted right before."""
    for fn in nc.m.functions:
        for blk in fn.blocks:
            insts = blk.instructions
            out = []
            changed = False
            for inst in insts:
                si = inst.sync_info
                if si is not None and inst.opcode not in _NO_HOIST:
                    waits = list(si.on_wait)
                    if len(waits) > 1:
                        for j, w in enumerate(waits[:-1]):
                            nop = mybir.InstNoOp(name=f"{inst.name}-wfix{j}")
                            nop.engine = inst.engine
                            nop.sync_info = mybir.SyncInfo(on_wait=[w], on_update=[])
                            out.append(nop)
                        inst.sync_info = mybir.SyncInfo(
                            on_wait=[waits[-1]], on_update=list(si.on_update)
                        )
                        changed = True
                out.append(inst)
            if changed:
                blk.instructions = out


def _build(reps=1, fix_waits=True, n_chunks=SC, trim=True, masks=True):
    nc = bass.Bass("TRN2", target_bir_lowering=False, debug=False,
                   num_devices=N_CORES)

    xT = nc.dram_tensor("xT", [128, 8, S], BF16, kind="ExternalInput").ap()
    w = nc.dram_tensor("w", [128, 8, 768], BF16, kind="ExternalInput").ap()
    wout = nc.dram_tensor("wout", [128, 2, D], BF16, kind="ExternalInput").ap()
    bq = nc.dram_tensor("bq", [128, 4], F32, kind="ExternalInput").ap()
    bv = nc.dram_tensor("bv", [128, 256], F32, kind="ExternalInput").ap()
    cmask = nc.dram_tensor("cmask", [128, 128], BF16, kind="ExternalInput").ap()
    outT = nc.dram_tensor("outT", [128, 8, S], BF16, kind="ExternalOutput").ap()

    with tile.TileContext(nc) as tc, ExitStack() as ctx:
        persist = ctx.enter_context(tc.tile_pool(name="persist", bufs=1))
        xpool = ctx.enter_context(tc.tile_pool(name="xp", bufs=3))
        epool = ctx.enter_context(tc.tile_pool(name="ep", bufs=3))
        rpool = ctx.enter_context(tc.tile_pool(name="rp", bufs=2))
        opool = ctx.enter_context(tc.tile_pool(name="op", bufs=4))
        # PSUM: scores 2x[128,1024] (4 banks) + AV accum 2x[128,512] (2) +
        # shared qkv/outproj/rb pool 2x[128,512] (2) = 8 banks
        ps_sc = ctx.enter_context(tc.tile_pool(name="ps_sc", bufs=2, space="PSUM"))
        ps_av = ctx.enter_context(tc.tile_pool(name="ps_av", bufs=2, space="PSUM"))
        ps_mm = ctx.enter_context(tc.tile_pool(name="ps_mm", bufs=2, space="PSUM"))

        w_sb = persist.tile([128, 8, 768], BF16, tag="w")
        wout_sb = persist.tile([128, 2, D], BF16, tag="wout")
        bq_sb = persist.tile([128, 4], F32, tag="bq")
        bv_sb = persist.tile([128, 256], F32, tag="bv")
        sel_sb = persist.tile([128, 256], BF16, tag="sel")
        cmask_sb = persist.tile([128, 128], BF16, tag="cmask")
        qT = persist.tile([128, 2, S], BF16, tag="qT")
        kT = persist.tile([128, 2, S], BF16, tag="kT")
        vn = persist.tile([128, KT, HL, 97], BF16, tag="vn")
        vraw = persist.tile([128, 2, S], BF16, tag="vraw")
        vnorm = persist.tile([128, 2, S], BF16, tag="vnorm")

        # device-built constants: the softmax-denominator ones column of the
        # augmented V — col 64 for even heads (den -> po row 64), col 96 for
        # odd heads (den -> po row 96, a valid mod-32 partition base) — and
        # the K=33 reciprocal-broadcast selector (rows 65..95 all zero).
        nc.vector.memset(sel_sb[64:96, 0:128], 0.0)
        nc.vector.memset(sel_sb[64:65, 0:64], 1.0)
        nc.vector.memset(sel_sb[96:97, 0:64], 0.0)
        nc.vector.memset(sel_sb[96:97, 64:128], 1.0)
        for h in range(HL):
            if h % 2 == 0:
                nc.vector.memset(vn[:, :, h, 64:65], 1.0)
            else:
                nc.vector.memset(vn[:, :, h, 64:96], 0.0)
                nc.vector.memset(vn[:, :, h, 96:97], 1.0)
        recrs = [persist.tile([128, 512], BF16, tag=f"recr{i}", name=f"recr{i}")
                 for i in range(2)]
        for r in recrs:
            nc.vector.memset(r[64:96, :], 0.0)

        # ---- startup DMA, k-sliced so the first matmuls start early ----
        xc0 = xpool.tile([128, 8, 512], BF16, tag="xc", name="xc0")
        nc.sync.dma_start(xc0[:, 0, :], xT[:, 0, 0:512])
        nc.scalar.dma_start(w_sb[:, 0, 0:512], w[:, 0, 0:512])
        nc.sync.dma_start(xc0[:, 1, :], xT[:, 1, 0:512])
        nc.scalar.dma_start(w_sb[:, 1, 0:512], w[:, 1, 0:512])
        for k2 in range(1, 4):
            ks2 = slice(2 * k2, 2 * k2 + 2)
            nc.sync.dma_start(xc0[:, ks2, :], xT[:, ks2, 0:512])
            nc.scalar.dma_start(w_sb[:, ks2, 0:512], w[:, ks2, 0:512])
        # V-projection columns arrive after the qk k-loop no longer needs the
        # DMA engines; the sequential v-blocks start right as these land
        nc.scalar.dma_start(w_sb[:, 0:4, 512:768], w[:, 0:4, 512:768])
        nc.scalar.dma_start(w_sb[:, 4:8, 512:768], w[:, 4:8, 512:768])
        nc.scalar.dma_start(bq_sb[:], bq)
        nc.scalar.dma_start(bv_sb[:], bv)
        nc.scalar.dma_start(cmask_sb[:], cmask)
        nc.scalar.dma_start(wout_sb[:], wout)

        # ---- PE clock warm-up: junk matmuls over the (memset) selector rows
        # while the first x/w slices stream in, so the p-state ramp completes
        # before the first real matmul (the result bank is never read)
        warm = ps_mm.tile([128, 512], F32, tag="mm", name="warm")
        for i in range(14):
            nc.tensor.matmul(warm[:, 0:256], sel_sb[64:96, 0:128],
                             sel_sb[64:96, 0:256], start=True, stop=True)

        # ---- chunk-0 qkv projection, k-outer (consumes slices as they land)
        # spA: [Q hp0 | K hp0], spB: [Q hp1 | K hp1]; pvA: [j0|j1], pvB: [j2|j3]
        # (hardware: at most ONE open matmul accumulation group per PSUM bank
        # — interleaved groups in a shared bank silently corrupt, so the four
        # qk accumulators get a bank each and V runs as sequential j-blocks)
        spA = ps_sc.tile([128, 1024], F32, tag="s", name="spA")
        spB = ps_sc.tile([128, 1024], F32, tag="s", name="spB")
        for k in range(8):
            st = k == 0
            sp_ = k == 7
            nc.tensor.matmul(spA[:, 0:512], w_sb[:, k, 0:128], xc0[:, k, :],
                             start=st, stop=sp_)
            nc.tensor.matmul(spA[:, 512:1024], w_sb[:, k, 256:384], xc0[:, k, :],
                             start=st, stop=sp_)
            nc.tensor.matmul(spB[:, 0:512], w_sb[:, k, 128:256], xc0[:, k, :],
                             start=st, stop=sp_)
            nc.tensor.matmul(spB[:, 512:1024], w_sb[:, k, 384:512], xc0[:, k, :],
                             start=st, stop=sp_)
        nc.vector.tensor_scalar_add(qT[:, 0, 0:512], spA[:, 0:512], bq_sb[:, 0:1])
        nc.vector.tensor_scalar_add(kT[:, 0, 0:512], spA[:, 512:1024], bq_sb[:, 2:3])
        nc.vector.tensor_scalar_add(qT[:, 1, 0:512], spB[:, 0:512], bq_sb[:, 1:2])
        nc.vector.tensor_scalar_add(kT[:, 1, 0:512], spB[:, 512:1024], bq_sb[:, 3:4])
        for st4 in range(4):
            pv = ps_av.tile([128, 512], F32, tag="av", name=f"pv0_{st4}")
            for k in range(8):
                nc.tensor.matmul(pv[:, 0:256],
                                 xc0[:, k, 128 * st4:128 * (st4 + 1)],
                                 w_sb[:, k, 512:768], start=(k == 0), stop=(k == 7))
            nc.vector.tensor_add(
                vn[:, st4, :, 0:64],
                pv[:, 0:256].rearrange("p (h d) -> p h d", h=4),
                bv_sb[:].rearrange("p (h d) -> p h d", h=4))

        xcs = {0: xc0}

        # ---- filler blocks: qkv projection of a later chunk / output
        # projection of an earlier chunk, emitted between attention steps ----
        def qk_halves(cn, m):
            # m: 0=Q hp0, 1=Q hp1, 2=K hp0, 3=K hp1 (matches w col + bias col)
            # split into two 4-k-step closures for finer filler granularity
            hold = {}

            def emit_a():
                hold["pm"] = ps_mm.tile([128, 512], F32, tag="mm",
                                        name=f"qk{cn}_{m}")
                for k in range(4):
                    nc.tensor.matmul(hold["pm"][:], w_sb[:, k, 128 * m:128 * (m + 1)],
                                     xcs[cn][:, k, :], start=(k == 0), stop=False)

            def emit_b():
                qs = slice(cn * 512, (cn + 1) * 512)
                pm = hold["pm"]
                for k in range(4, 8):
                    nc.tensor.matmul(pm[:], w_sb[:, k, 128 * m:128 * (m + 1)],
                                     xcs[cn][:, k, :], start=False, stop=(k == 7))
                dst = qT[:, m, qs] if m < 2 else kT[:, m - 2, qs]
                nc.vector.tensor_scalar_add(dst, pm[:], bq_sb[:, m:m + 1])
            return [emit_a, emit_b]

        def v_block(cn, j):
            def emit():
                pv = ps_mm.tile([128, 512], F32, tag="mm", name=f"v{cn}_{j}")
                for k in range(8):
                    nc.tensor.matmul(pv[:, 0:256],
                                     xcs[cn][:, k, 128 * j:128 * (j + 1)],
                                     w_sb[:, k, 512:768], start=(k == 0), stop=(k == 7))
                st4 = 4 * cn + j
                nc.vector.tensor_add(
                    vn[:, st4, :, 0:64],
                    pv[:, 0:256].rearrange("p (h d) -> p h d", h=4),
                    bv_sb[:].rearrange("p (h d) -> p h d", h=4))
            return emit

        ou_hold = {}
        pu_hold = {}

        def outproj_block(cn, m, copy_eng="dve", psum="mm"):
            # even m allocates a 2-block staging tile; odd m completes it and
            # issues one paired DMA (halves the per-transfer HWDGE overhead).
            # psum="sc": after the last scores, the 4 score banks are free —
            # pair two blocks per [128,1024] tile for deeper PU buffering.
            def emit():
                qs = slice(cn * 512, (cn + 1) * 512)
                if psum == "mm":
                    pu = ps_mm.tile([128, 512], F32, tag="mm",
                                    name=f"pu{cn}_{m}")[:]
                else:
                    if m % 2 == 0:
                        pu_hold[cn] = ps_sc.tile([128, 1024], F32, tag="s",
                                                 name=f"pu2{cn}_{m}")
                    pu = pu_hold[cn][:, 512 * (m % 2):512 * (m % 2) + 512]
                for t in range(2):
                    nc.tensor.matmul(pu, wout_sb[:, t, 128 * m:128 * (m + 1)],
                                     vnorm[:, t, qs], start=(t == 0), stop=(t == 1))
                if m % 2 == 0:
                    ou_hold[cn] = opool.tile([128, 2, 512], BF16, tag="ou",
                                             name=f"ou{cn}_{m}")
                ou = ou_hold[cn]
                dst = ou[:, m % 2, :]
                if copy_eng == "dve":
                    nc.vector.tensor_copy(dst, pu)
                elif copy_eng == "act":
                    nc.scalar.copy(dst, pu)
                if m % 2 == 1:
                    nc.sync.dma_start(outT[:, m - 1:m + 1, qs], ou[:])
            return emit

        # pending normalization chain of the previous head pair — emitted
        # right AFTER the next head pair's first exp is queued, so the Act
        # engine starts the next exp before the drain copies, and the PE has
        # scores/filler work while the reciprocal chain flows (crosses chunk
        # boundaries too)
        pending = {"norm": None}

        # ---- attention for one q-chunk, with filler drained between steps
        def attn_qc(qc, fillers, reserve=()):
            n_ki = 4 * qc + 4
            nsteps = 2 * n_ki + 2
            state = {"step": 0, "drained": 0}

            def tick(n=None):
                state["step"] += 1
                if n is None:
                    # at least one filler by step 1: right after a head-pair
                    # boundary the PE otherwise idles on the exp/recip chains
                    target = max(len(fillers) * state["step"] // nsteps,
                                 min(2, state["step"]))
                else:
                    target = state["drained"] + n
                while state["drained"] < min(target, len(fillers)):
                    fillers[state["drained"]]()
                    state["drained"] += 1

            qs = slice(qc * 512, (qc + 1) * 512)
            for hp in range(2):
                # po tiles are allocated lazily at the first AV so the pool
                # WAR lands after the previous pair's (deferred) drain copies
                po = [None, None]
                recr = recrs[(2 * qc + hp) % 2]
                es_hold = [None] * n_ki

                def do_av(ki, qc=qc, hp=hp, po=po, n_ki=n_ki, es_hold=es_hold):
                    if po[0] is None:
                        po[0] = ps_av.tile([128, 512], F32, tag="av",
                                           name=f"po{qc}{hp}0")
                        po[1] = ps_av.tile([128, 512], F32, tag="av",
                                           name=f"po{qc}{hp}1")
                    e, o, wdt = es_hold[ki]
                    for i in range(2):
                        h = 2 * hp + i
                        # V_aug.T @ E: rows 0..63 values, row 64 (even) or 96
                        # (odd) the softmax denominator (ones column of V_aug)
                        nc.tensor.matmul(
                            po[i][0:65 + 32 * i, o:512], vn[:, ki, h, 0:65 + 32 * i],
                            e[:, i * 512:i * 512 + wdt],
                            start=(ki == 0), stop=(ki == n_ki - 1),
                            skip_group_check=True)

                for ki in range(n_ki):
                    j = ki - 4 * qc
                    o = 128 * j if (trim and j >= 0) else 0
                    wdt = 512 - o
                    ks = slice(ki * 128, (ki + 1) * 128)
                    qsub = slice(qc * 512 + o, (qc + 1) * 512)
                    # head slabs live at bank-aligned offsets i*512 — the two
                    # tile_position score groups must not share a PSUM bank
                    sp = ps_sc.tile([128, 1024], F32, tag="s",
                                    name=f"sp{qc}{hp}{ki}")
                    for i in range(2):   # head within pair (row-packed)
                        vp = 64 * i
                        nc.tensor.matmul(
                            sp[:, i * 512:i * 512 + wdt],
                            kT[vp:vp + 64, hp, ks], qT[vp:vp + 64, hp, qsub],
                            start=True, stop=True, tile_position=(vp, 0))
                    e = epool.tile([128, 1024], BF16, tag="e",
                                   name=f"e{qc}{hp}{ki}")
                    if wdt == 512:
                        nc.scalar.activation(e[:], sp[:], EXP, scale=0.125)
                    else:
                        sp3 = sp[:].rearrange("p (t q) -> p t q", t=2)
                        e3 = e[:].rearrange("p (t q) -> p t q", t=2)
                        nc.scalar.activation(e3[:, :, 0:wdt], sp3[:, :, 0:wdt],
                                             EXP, scale=0.125)
                    if masks and j >= 0:
                        # diagonal tile: with o=128j the invalid region is
                        # always the leading 128-col triangle (kk > qq)
                        mw = min(128, wdt)
                        for i in range(2):
                            es = e[:, i * 512:i * 512 + mw]
                            nc.vector.tensor_mul(es, es, cmask_sb[:, 0:mw])
                    es_hold[ki] = (e, o, wdt)
                    if ki == 0 and pending["norm"] is not None:
                        pending["norm"]()
                        pending["norm"] = None
                    if ki >= 1:
                        do_av(ki - 1)
                    tick()
                do_av(n_ki - 1)
                tick()

                # normalization: per-head reciprocal of the denominator rows,
                # broadcast onto value partitions via one K=33 selector matmul.
                # Denominators: even head on po[0] row 64, odd head on po[1]
                # row 96 (the odd V_aug ones column sits at col 96), so both
                # reciprocals stay partition-aligned (rows 65..95 are zero).
                def norm(qc=qc, hp=hp, po=po, recr=recr, qs=qs,
                         res=tuple(reserve) if (hp == 1 and reserve) else ()):
                    stage = rpool.tile([128, 512], BF16, tag="stage",
                                       name=f"st{qc}{hp}")
                    nc.scalar.copy(stage[0:64, :], po[1][0:64, :])
                    # sync ring: out-DMAs queued behind this were emitted in
                    # the same window, so the short stage-copy wait cannot
                    # head-of-line-block them for long
                    nc.sync.dma_start(vraw[64:128, hp, qs], stage[0:64, :])
                    with nc.allow_low_precision(reason="bf16 softmax recip"):
                        nc.vector.reciprocal(recr[96:97, :], po[1][96:97, :])
                        nc.vector.reciprocal(recr[64:65, :], po[0][64:65, :])
                    nc.scalar.copy(vraw[0:64, hp, qs], po[0][0:64, :])
                    if res:
                        res[0]()
                        res[1]()
                    else:
                        tick(n=2)
                    rb = ps_mm.tile([128, 512], F32, tag="mm", name=f"rb{qc}{hp}")
                    nc.tensor.matmul(rb[:], sel_sb[64:97, 0:128], recr[64:97, :],
                                     start=True, stop=True, tile_position=(64, 0))
                    for r in res[2:]:
                        r()
                    nc.vector.tensor_mul(vnorm[:, hp, qs], vraw[:, hp, qs], rb[:])
                pending["norm"] = norm
            # any fillers not yet drained
            while state["drained"] < len(fillers):
                fillers[state["drained"]]()
                state["drained"] += 1

        # ---- main schedule: attn(c) with qkv(c+1) as filler; all deferrable
        # output projections (chunks 0..2) land in attn(3), whose exp cadence
        # otherwise starves the tensor engine; outproj(3) is the tail.  In
        # attn(3) the copies ride the idle Pool engine so the DVE recip ->
        # vnorm chain stays short; the last few blocks are reserved to keep
        # the PE warm through the final normalization chain.
        for c in range(n_chunks):
            fillers, reserve = [], []
            if c + 1 < n_chunks:
                xc = xpool.tile([128, 8, 512], BF16, tag="xc", name=f"xc{c+1}")
                qsn = slice((c + 1) * 512, (c + 2) * 512)
                nc.sync.dma_start(xc[:, 0:4, :], xT[:, 0:4, qsn])
                nc.sync.dma_start(xc[:, 4:8, :], xT[:, 4:8, qsn])
                xcs[c + 1] = xc
                for m in range(4):
                    fillers += qk_halves(c + 1, m)
                fillers += [v_block(c + 1, j) for j in range(4)]
            else:
                # GPSIMD cannot touch PSUM, so drain copies ride DVE (fillers)
                # and Act (reserve, where the exp pipeline has already drained)
                for cn in range(n_chunks - 2):
                    fillers += [outproj_block(cn, m, "dve") for m in range(8)]
                fillers += [outproj_block(n_chunks - 2, m, "dve") for m in range(4)]
                reserve = [outproj_block(n_chunks - 2, m, "act") for m in range(4, 8)]
            attn_qc(c, fillers, reserve)
        # the last head pair's normalization, then the tail output projection
        pending["norm"]()
        pending["norm"] = None
        for m in range(8):
            outproj_block(n_chunks - 1, m, copy_eng=("dve" if m % 2 else "act"))()

    if fix_waits:
        _fix_sync_waits(nc)
    return nc


def _get_nc():
    if "nc" not in _CACHE:
        _CACHE["nc"] = _build()
    return _CACHE["nc"]


def _make_cmask() -> np.ndarray:
    """cmask[128, 128]: c[kk, qq] = 1 iff kk <= qq (relative causal triangle
    applied to the leading 128 cols of every diagonal score tile)."""
    kk = np.arange(128)[:, None]
    qq = np.arange(128)[None, :]
    return (kk <= qq).astype(np.float32)


def kernel(x, W_qkv, b_qkv, W_out, b_out):
    x = np.asarray(x, np.float32)
    W_qkv = np.asarray(W_qkv, np.float32)
    b_qkv = np.asarray(b_qkv, np.float32)
    W_out = np.asarray(W_out, np.float32)
    b_out = np.asarray(b_out, np.float32)

    nc = _get_nc()
    cmask = _make_cmask().astype(ml_dtypes.bfloat16)

    in_maps = []
    for c in range(N_CORES):
        b, g = divmod(c, 4)
        heads = [4 * g + i for i in range(HL)]
        # reorder W_qkv columns: [Q(h0..h3) | K(h0..h3) | V(h0..h3)]
        qcols = np.concatenate([W_qkv[:, h * 192:h * 192 + 64] for h in heads], 1)
        kcols = np.concatenate([W_qkv[:, h * 192 + 64:h * 192 + 128] for h in heads], 1)
        vcols = np.concatenate([W_qkv[:, h * 192 + 128:h * 192 + 192] for h in heads], 1)
        wsh = np.concatenate([qcols, kcols, vcols], 1)          # [1024, 768]
        bqv = np.concatenate([b_qkv[h * 192:h * 192 + 64] for h in heads])
        bkv = np.concatenate([b_qkv[h * 192 + 64:h * 192 + 128] for h in heads])
        bvv = np.concatenate([b_qkv[h * 192 + 128:h * 192 + 192] for h in heads])
        wo = W_out[g * 256:(g + 1) * 256, :]                    # [256, 1024]

        xT = x[b].T.reshape(8, 128, S).transpose(1, 0, 2)       # [128, 8, S]
        wsh3 = wsh.reshape(8, 128, 768).transpose(1, 0, 2)      # [128, 8, 768]
        wo3 = wo.reshape(2, 128, D).transpose(1, 0, 2)          # [128, 2, D]
        bq2 = np.concatenate([bqv, bkv]).reshape(4, 128).T      # [128, 4]
        bv2 = np.broadcast_to(bvv, (128, 256))                  # [128, 256]

        in_maps.append({
            "xT": np.ascontiguousarray(xT).astype(ml_dtypes.bfloat16),
            "w": np.ascontiguousarray(wsh3).astype(ml_dtypes.bfloat16),
            "wout": np.ascontiguousarray(wo3).astype(ml_dtypes.bfloat16),
            "bq": np.ascontiguousarray(bq2),
            "bv": np.ascontiguousarray(bv2),
            "cmask": np.ascontiguousarray(cmask),
        })

    _CACHE["in_maps"] = in_maps
    res = bass_utils.run_bass_kernel_spmd(nc, in_maps, core_ids=list(range(N_CORES)))

    out = np.zeros((B, S, D), np.float32)
    for c in range(N_CORES):
        b = c // 4
        oT = np.asarray(res.results[c]["outT"]).astype(np.float32)  # [128, 8, S]
        out[b] += oT.transpose(1, 0, 2).reshape(D, S).T
    out += b_out
    return out


# revision 84
# speedup vs baseline: 1.0149x; 1.0115x over previous
"""Multi-head causal attention (B=2, S=2048, D=1024, H=16) on 8 TRN2 NeuronCores.

Sharding: core c in 0..7 handles batch b = c // 4 and local head group
g = c % 4 (global heads 4g .. 4g+3).  Tensor-parallel over heads: each core
computes its heads' Q/K/V projections, causal attention, and a partial
output projection (W_out rows for its heads).  Host sums the 4 partials per
batch and adds b_out.

v2: bf16 storage everywhere (f32 PSUM accumulation), 128-granularity causal
trim, k-sliced startup DMA with k-outer chunk-0 projection, and phase
interleaving: qkv(c+1) / outproj(c-1) matmul blocks are emitted as filler
between attention ki-steps of chunk c so the tensor engine never waits on
the activation-engine exp cadence.
"""

from contextlib import ExitStack

import numpy as np
import ml_dtypes

import concourse.bass as bass
import concourse.mybir as mybir
import concourse.tile as tile
from concourse import bass_utils

F32 = mybir.dt.float32
BF16 = mybir.dt.bfloat16
EXP = mybir.ActivationFunctionType.Exp

B, S, D, H = 2, 2048, 1024, 16
HD = D // H          # 64
HL = 4               # heads per core
N_CORES = 8
SC = S // 512        # 4 q-chunks of 512
KT = S // 128        # 16 k-tiles of 128

_CACHE = {}

_NO_HOIST = {
    "AllEngineBarrier",
    "EventSemaphore",
    "UnconditionalBranch",
    "CompareAndBranch",
    "BranchHint",
    "IndirectBranch",
    "Halt",
    "Call",
    "OverlayCall",
    "NoOp",
}


def _fix_sync_waits(nc):
    """walrus codegen holds only one sync-wait per engine instruction; hoist
    excess waits onto same-engine NoOps inserted right before."""
    for fn in nc.m.functions:
        for blk in fn.blocks:
            insts = blk.instructions
            out = []
            changed = False
            for inst in insts:
                si = inst.sync_info
                if si is not None and inst.opcode not in _NO_HOIST:
                    waits = list(si.on_wait)
                    if len(waits) > 1:
                        for j, w in enumerate(waits[:-1]):
                            nop = mybir.InstNoOp(name=f"{inst.name}-wfix{j}")
                            nop.engine = inst.engine
                            nop.sync_info = mybir.SyncInfo(on_wait=[w], on_update=[])
                            out.append(nop)
                        inst.sync_info = mybir.SyncInfo(
                            on_wait=[waits[-1]], on_update=list(si.on_update)
                        )
                        changed = True
                out.append(inst)
            if changed:
                blk.instructions = out


def _build(reps=1, fix_waits=True, n_chunks=SC, trim=True, masks=True):
    nc = bass.Bass("TRN2", target_bir_lowering=False, debug=False,
                   num_devices=N_CORES)

    xT = nc.dram_tensor("xT", [128, 8, S], BF16, kind="ExternalInput").ap()
    w = nc.dram_tensor("w", [128, 8, 768], BF16, kind="ExternalInput").ap()
    wout = nc.dram_tensor("wout", [128, 2, D], BF16, kind="ExternalInput").ap()
    bq = nc.dram_tensor("bq", [128, 4], F32, kind="ExternalInput").ap()
    bv = nc.dram_tensor("bv", [128, 256], F32, kind="ExternalInput").ap()
    cmask = nc.dram_tensor("cmask", [128, 128], BF16, kind="ExternalInput").ap()
    outT = nc.dram_tensor("outT", [128, 8, S], BF16, kind="ExternalOutput").ap()

    with tile.TileContext(nc) as tc, ExitStack() as ctx:
        persist = ctx.enter_context(tc.tile_pool(name="persist", bufs=1))
        xpool = ctx.enter_context(tc.tile_pool(name="xp", bufs=3))
        epool = ctx.enter_context(tc.tile_pool(name="ep", bufs=3))
        rpool = ctx.enter_context(tc.tile_pool(name="rp", bufs=2))
        opool = ctx.enter_context(tc.tile_pool(name="op", bufs=4))
        # PSUM: scores 2x[128,1024] (4 banks) + AV accum 2x[128,512] (2) +
        # shared qkv/outproj/rb pool 2x[128,512] (2) = 8 banks
        ps_sc = ctx.enter_context(tc.tile_pool(name="ps_sc", bufs=2, space="PSUM"))
        ps_av = ctx.enter_context(tc.tile_pool(name="ps_av", bufs=2, space="PSUM"))
        ps_mm = ctx.enter_context(tc.tile_pool(name="ps_mm", bufs=2, space="PSUM"))

        w_sb = persist.tile([128, 8, 768], BF16, tag="w")
        wout_sb = persist.tile([128, 2, D], BF16, tag="wout")
        bq_sb = persist.tile([128, 4], F32, tag="bq")
        bv_sb = persist.tile([128, 256], F32, tag="bv")
        sel_sb = persist.tile([128, 256], BF16, tag="sel")
        cmask_sb = persist.tile([128, 128], BF16, tag="cmask")
        qT = persist.tile([128, 2, S], BF16, tag="qT")
        kT = persist.tile([128, 2, S], BF16, tag="kT")
        vn = persist.tile([128, KT, HL, 97], BF16, tag="vn")
        vraw = persist.tile([128, 2, S], BF16, tag="vraw")
        vnorm = persist.tile([128, 2, S], BF16, tag="vnorm")

        # device-built constants: the softmax-denominator ones column of the
        # augmented V — col 64 for even heads (den -> po row 64), col 96 for
        # odd heads (den -> po row 96, a valid mod-32 partition base) — and
        # the K=33 reciprocal-broadcast selector (rows 65..95 all zero).
        nc.vector.memset(sel_sb[64:96, 0:128], 0.0)
        nc.vector.memset(sel_sb[64:65, 0:64], 1.0)
        nc.vector.memset(sel_sb[96:97, 0:64], 0.0)
        nc.vector.memset(sel_sb[96:97, 64:128], 1.0)
        for h in range(HL):
            if h % 2 == 0:
                nc.vector.memset(vn[:, :, h, 64:65], 1.0)
            else:
                nc.vector.memset(vn[:, :, h, 64:96], 0.0)
                nc.vector.memset(vn[:, :, h, 96:97], 1.0)
        recrs = [persist.tile([128, 512], BF16, tag=f"recr{i}", name=f"recr{i}")
                 for i in range(2)]
        for r in recrs:
            nc.vector.memset(r[64:96, :], 0.0)

        # ---- startup DMA, k-sliced so the first matmuls start early ----
        xc0 = xpool.tile([128, 8, 512], BF16, tag="xc", name="xc0")
        nc.sync.dma_start(xc0[:, 0, :], xT[:, 0, 0:512])
        nc.scalar.dma_start(w_sb[:, 0, 0:512], w[:, 0, 0:512])
        nc.sync.dma_start(xc0[:, 1, :], xT[:, 1, 0:512])
        nc.scalar.dma_start(w_sb[:, 1, 0:512], w[:, 1, 0:512])
        for k2 in range(1, 4):
            ks2 = slice(2 * k2, 2 * k2 + 2)
            nc.sync.dma_start(xc0[:, ks2, :], xT[:, ks2, 0:512])
            nc.scalar.dma_start(w_sb[:, ks2, 0:512], w[:, ks2, 0:512])
        # V-projection columns arrive after the qk k-loop no longer needs the
        # DMA engines; the sequential v-blocks start right as these land
        nc.scalar.dma_start(w_sb[:, 0:4, 512:768], w[:, 0:4, 512:768])
        nc.scalar.dma_start(w_sb[:, 4:8, 512:768], w[:, 4:8, 512:768])
        nc.scalar.dma_start(bq_sb[:], bq)
        nc.scalar.dma_start(bv_sb[:], bv)
        nc.scalar.dma_start(cmask_sb[:], cmask)
        nc.scalar.dma_start(wout_sb[:], wout)

        # ---- PE clock warm-up: junk matmuls over the (memset) selector rows
        # while the first x/w slices stream in, so the p-state ramp completes
        # before the first real matmul (the result bank is never read)
        warm = ps_mm.tile([128, 512], F32, tag="mm", name="warm")
        for i in range(14):
            nc.tensor.matmul(warm[:, 0:256], sel_sb[64:96, 0:128],
                             sel_sb[64:96, 0:256], start=True, stop=True)

        # ---- chunk-0 qkv projection, k-outer (consumes slices as they land)
        # spA: [Q hp0 | K hp0], spB: [Q hp1 | K hp1]; pvA: [j0|j1], pvB: [j2|j3]
        # (hardware: at most ONE open matmul accumulation group per PSUM bank
        # — interleaved groups in a shared bank silently corrupt, so the four
        # qk accumulators get a bank each and V runs as sequential j-blocks)
        spA = ps_sc.tile([128, 1024], F32, tag="s", name="spA")
        spB = ps_sc.tile([128, 1024], F32, tag="s", name="spB")
        for k in range(8):
            st = k == 0
            sp_ = k == 7
            nc.tensor.matmul(spA[:, 0:512], w_sb[:, k, 0:128], xc0[:, k, :],
                             start=st, stop=sp_)
            nc.tensor.matmul(spA[:, 512:1024], w_sb[:, k, 256:384], xc0[:, k, :],
                             start=st, stop=sp_)
            nc.tensor.matmul(spB[:, 0:512], w_sb[:, k, 128:256], xc0[:, k, :],
                             start=st, stop=sp_)
            nc.tensor.matmul(spB[:, 512:1024], w_sb[:, k, 384:512], xc0[:, k, :],
                             start=st, stop=sp_)
        nc.vector.tensor_scalar_add(qT[:, 0, 0:512], spA[:, 0:512], bq_sb[:, 0:1])
        nc.vector.tensor_scalar_add(kT[:, 0, 0:512], spA[:, 512:1024], bq_sb[:, 2:3])
        nc.vector.tensor_scalar_add(qT[:, 1, 0:512], spB[:, 0:512], bq_sb[:, 1:2])
        nc.vector.tensor_scalar_add(kT[:, 1, 0:512], spB[:, 512:1024], bq_sb[:, 3:4])
        for st4 in range(4):
            pv = ps_av.tile([128, 512], F32, tag="av", name=f"pv0_{st4}")
            for k in range(8):
                nc.tensor.matmul(pv[:, 0:256],
                                 xc0[:, k, 128 * st4:128 * (st4 + 1)],
                                 w_sb[:, k, 512:768], start=(k == 0), stop=(k == 7))
            nc.vector.tensor_add(
                vn[:, st4, :, 0:64],
                pv[:, 0:256].rearrange("p (h d) -> p h d", h=4),
                bv_sb[:].rearrange("p (h d) -> p h d", h=4))

        xcs = {0: xc0}

        # ---- filler blocks: qkv projection of a later chunk / output
        # projection of an earlier chunk, emitted between attention steps ----
        def qk_halves(cn, m):
            # m: 0=Q hp0, 1=Q hp1, 2=K hp0, 3=K hp1 (matches w col + bias col)
            # split into two 4-k-step closures for finer filler granularity
            hold = {}

            def emit_a():
                hold["pm"] = ps_mm.tile([128, 512], F32, tag="mm",
                                        name=f"qk{cn}_{m}")
                for k in range(4):
                    nc.tensor.matmul(hold["pm"][:], w_sb[:, k, 128 * m:128 * (m + 1)],
                                     xcs[cn][:, k, :], start=(k == 0), stop=False)

            def emit_b():
                qs = slice(cn * 512, (cn + 1) * 512)
                pm = hold["pm"]
                for k in range(4, 8):
                    nc.tensor.matmul(pm[:], w_sb[:, k, 128 * m:128 * (m + 1)],
                                     xcs[cn][:, k, :], start=False, stop=(k == 7))
                dst = qT[:, m, qs] if m < 2 else kT[:, m - 2, qs]
                nc.vector.tensor_scalar_add(dst, pm[:], bq_sb[:, m:m + 1])
            return [emit_a, emit_b]

        def v_block(cn, j):
            def emit():
                pv = ps_mm.tile([128, 512], F32, tag="mm", name=f"v{cn}_{j}")
                for k in range(8):
                    nc.tensor.matmul(pv[:, 0:256],
                                     xcs[cn][:, k, 128 * j:128 * (j + 1)],
                                     w_sb[:, k, 512:768], start=(k == 0), stop=(k == 7))
                st4 = 4 * cn + j
                nc.vector.tensor_add(
                    vn[:, st4, :, 0:64],
                    pv[:, 0:256].rearrange("p (h d) -> p h d", h=4),
                    bv_sb[:].rearrange("p (h d) -> p h d", h=4))
            return emit

        ou_hold = {}
        pu_hold = {}

        def outproj_block(cn, m, copy_eng="dve", psum="mm"):
            # even m allocates a 2-block staging tile; odd m completes it and
            # issues one paired DMA (halves the per-transfer HWDGE overhead).
            # psum="sc": after the last scores, the 4 score banks are free —
            # pair two blocks per [128,1024] tile for deeper PU buffering.
            def emit():
                qs = slice(cn * 512, (cn + 1) * 512)
                if psum == "mm":
                    pu = ps_mm.tile([128, 512], F32, tag="mm",
                                    name=f"pu{cn}_{m}")[:]
                elif psum == "alt":
                    # tail: the AV banks are free — alternate pools so four
                    # blocks can be in flight instead of two
                    pool_ = ps_mm if m % 2 == 0 else ps_av
                    pu = pool_.tile([128, 512], F32, tag=("mm" if m % 2 == 0 else "av"),
                                    name=f"pu{cn}_{m}")[:]
                else:
                    if m % 2 == 0:
                        pu_hold[cn] = ps_sc.tile([128, 1024], F32, tag="s",
                                                 name=f"pu2{cn}_{m}")
                    pu = pu_hold[cn][:, 512 * (m % 2):512 * (m % 2) + 512]
                for t in range(2):
                    nc.tensor.matmul(pu, wout_sb[:, t, 128 * m:128 * (m + 1)],
                                     vnorm[:, t, qs], start=(t == 0), stop=(t == 1))
                if m % 2 == 0:
                    ou_hold[cn] = opool.tile([128, 2, 512], BF16, tag="ou",
                                             name=f"ou{cn}_{m}")
                ou = ou_hold[cn]
                dst = ou[:, m % 2, :]
                if copy_eng == "dve":
                    nc.vector.tensor_copy(dst, pu)
                elif copy_eng == "act":
                    nc.scalar.copy(dst, pu)
                if m % 2 == 1:
                    nc.sync.dma_start(outT[:, m - 1:m + 1, qs], ou[:])
            return emit

        # pending normalization chain of the previous head pair — emitted
        # right AFTER the next head pair's first exp is queued, so the Act
        # engine starts the next exp before the drain copies, and the PE has
        # scores/filler work while the reciprocal chain flows (crosses chunk
        # boundaries too)
        pending = {"norm": None}

        # ---- attention for one q-chunk, with filler drained between steps
        def attn_qc(qc, fillers, reserve=()):
            n_ki = 4 * qc + 4
            nsteps = 2 * n_ki + 2
            state = {"step": 0, "drained": 0}

            def tick(n=None):
                state["step"] += 1
                if n is None:
                    # at least one filler by step 1: right after a head-pair
                    # boundary the PE otherwise idles on the exp/recip chains
                    target = max(len(fillers) * state["step"] // nsteps,
                                 min(2, state["step"]))
                else:
                    target = state["drained"] + n
                while state["drained"] < min(target, len(fillers)):
                    fillers[state["drained"]]()
                    state["drained"] += 1

            qs = slice(qc * 512, (qc + 1) * 512)
            for hp in range(2):
                # po tiles are allocated lazily at the first AV so the pool
                # WAR lands after the previous pair's (deferred) drain copies
                po = [None, None]
                recr = recrs[(2 * qc + hp) % 2]
                es_hold = [None] * n_ki

                def do_av(ki, qc=qc, hp=hp, po=po, n_ki=n_ki, es_hold=es_hold):
                    if po[0] is None:
                        po[0] = ps_av.tile([128, 512], F32, tag="av",
                                           name=f"po{qc}{hp}0")
                        po[1] = ps_av.tile([128, 512], F32, tag="av",
                                           name=f"po{qc}{hp}1")
                    e, o, wdt = es_hold[ki]
                    for i in range(2):
                        h = 2 * hp + i
                        # V_aug.T @ E: rows 0..63 values, row 64 (even) or 96
                        # (odd) the softmax denominator (ones column of V_aug)
                        nc.tensor.matmul(
                            po[i][0:65 + 32 * i, o:512], vn[:, ki, h, 0:65 + 32 * i],
                            e[:, i * 512:i * 512 + wdt],
                            start=(ki == 0), stop=(ki == n_ki - 1),
                            skip_group_check=True)

                for ki in range(n_ki):
                    j = ki - 4 * qc
                    o = 128 * j if (trim and j >= 0) else 0
                    wdt = 512 - o
                    ks = slice(ki * 128, (ki + 1) * 128)
                    qsub = slice(qc * 512 + o, (qc + 1) * 512)
                    # head slabs live at bank-aligned offsets i*512 — the two
                    # tile_position score groups must not share a PSUM bank
                    sp = ps_sc.tile([128, 1024], F32, tag="s",
                                    name=f"sp{qc}{hp}{ki}")
                    for i in range(2):   # head within pair (row-packed)
                        vp = 64 * i
                        nc.tensor.matmul(
                            sp[:, i * 512:i * 512 + wdt],
                            kT[vp:vp + 64, hp, ks], qT[vp:vp + 64, hp, qsub],
                            start=True, stop=True, tile_position=(vp, 0))
                    e = epool.tile([128, 1024], BF16, tag="e",
                                   name=f"e{qc}{hp}{ki}")
                    if wdt == 512:
                        nc.scalar.activation(e[:], sp[:], EXP, scale=0.125)
                    else:
                        sp3 = sp[:].rearrange("p (t q) -> p t q", t=2)
                        e3 = e[:].rearrange("p (t q) -> p t q", t=2)
                        nc.scalar.activation(e3[:, :, 0:wdt], sp3[:, :, 0:wdt],
                                             EXP, scale=0.125)
                    if masks and j >= 0:
                        # diagonal tile: with o=128j the invalid region is
                        # always the leading 128-col triangle (kk > qq)
                        mw = min(128, wdt)
                        for i in range(2):
                            es = e[:, i * 512:i * 512 + mw]
                            nc.vector.tensor_mul(es, es, cmask_sb[:, 0:mw])
                    es_hold[ki] = (e, o, wdt)
                    if ki == 0 and pending["norm"] is not None:
                        pending["norm"]()
                        pending["norm"] = None
                    if ki >= 1:
                        do_av(ki - 1)
                    tick()
                do_av(n_ki - 1)
                tick()

                # normalization: per-head reciprocal of the denominator rows,
                # broadcast onto value partitions via one K=33 selector matmul.
                # Denominators: even head on po[0] row 64, odd head on po[1]
                # row 96 (the odd V_aug ones column sits at col 96), so both
                # reciprocals stay partition-aligned (rows 65..95 are zero).
                def norm(qc=qc, hp=hp, po=po, recr=recr, qs=qs,
                         res=tuple(reserve) if (hp == 1 and reserve) else ()):
                    stage = rpool.tile([128, 512], BF16, tag="stage",
                                       name=f"st{qc}{hp}")
                    nc.scalar.copy(stage[0:64, :], po[1][0:64, :])
                    # sync ring: out-DMAs queued behind this were emitted in
                    # the same window, so the short stage-copy wait cannot
                    # head-of-line-block them for long
                    nc.sync.dma_start(vraw[64:128, hp, qs], stage[0:64, :])
                    with nc.allow_low_precision(reason="bf16 softmax recip"):
                        nc.vector.reciprocal(recr[96:97, :], po[1][96:97, :])
                        nc.vector.reciprocal(recr[64:65, :], po[0][64:65, :])
                    nc.scalar.copy(vraw[0:64, hp, qs], po[0][0:64, :])
                    if res:
                        res[0]()
                        res[1]()
                    else:
                        tick(n=2)
                    rb = ps_mm.tile([128, 512], F32, tag="mm", name=f"rb{qc}{hp}")
                    nc.tensor.matmul(rb[:], sel_sb[64:97, 0:128], recr[64:97, :],
                                     start=True, stop=True, tile_position=(64, 0))
                    for r in res[2:]:
                        r()
                    nc.vector.tensor_mul(vnorm[:, hp, qs], vraw[:, hp, qs], rb[:])
                pending["norm"] = norm
            # any fillers not yet drained
            while state["drained"] < len(fillers):
                fillers[state["drained"]]()
                state["drained"] += 1

        # ---- main schedule: attn(c) with qkv(c+1) as filler; all deferrable
        # output projections (chunks 0..2) land in attn(3), whose exp cadence
        # otherwise starves the tensor engine; outproj(3) is the tail.  In
        # attn(3) the copies ride the idle Pool engine so the DVE recip ->
        # vnorm chain stays short; the last few blocks are reserved to keep
        # the PE warm through the final normalization chain.
        for c in range(n_chunks):
            fillers, reserve = [], []
            if c + 1 < n_chunks:
                xc = xpool.tile([128, 8, 512], BF16, tag="xc", name=f"xc{c+1}")
                qsn = slice((c + 1) * 512, (c + 2) * 512)
                nc.sync.dma_start(xc[:, 0:4, :], xT[:, 0:4, qsn])
                nc.sync.dma_start(xc[:, 4:8, :], xT[:, 4:8, qsn])
                xcs[c + 1] = xc
                for m in range(4):
                    fillers += qk_halves(c + 1, m)
                fillers += [v_block(c + 1, j) for j in range(4)]
            else:
                # GPSIMD cannot touch PSUM, so drain copies ride DVE (fillers)
                # and Act (reserve, where the exp pipeline has already drained)
                for cn in range(n_chunks - 2):
                    fillers += [outproj_block(cn, m, "dve") for m in range(8)]
                fillers += [outproj_block(n_chunks - 2, m, "dve") for m in range(4)]
                reserve = [outproj_block(n_chunks - 2, m, "act") for m in range(4, 8)]
            attn_qc(c, fillers, reserve)
        # the last head pair's normalization, then the tail output projection
        pending["norm"]()
        pending["norm"] = None
        for m in range(8):
            outproj_block(n_chunks - 1, m, copy_eng=("dve" if m % 2 else "act"),
                          psum="alt")()

    if fix_waits:
        _fix_sync_waits(nc)
    return nc


def _get_nc():
    if "nc" not in _CACHE:
        _CACHE["nc"] = _build()
    return _CACHE["nc"]


def _make_cmask() -> np.ndarray:
    """cmask[128, 128]: c[kk, qq] = 1 iff kk <= qq (relative causal triangle
    applied to the leading 128 cols of every diagonal score tile)."""
    kk = np.arange(128)[:, None]
    qq = np.arange(128)[None, :]
    return (kk <= qq).astype(np.float32)


def kernel(x, W_qkv, b_qkv, W_out, b_out):
    x = np.asarray(x, np.float32)
    W_qkv = np.asarray(W_qkv, np.float32)
    b_qkv = np.asarray(b_qkv, np.float32)
    W_out = np.asarray(W_out, np.float32)
    b_out = np.asarray(b_out, np.float32)

    nc = _get_nc()
    cmask = _make_cmask().astype(ml_dtypes.bfloat16)

    in_maps = []
    for c in range(N_CORES):
        b, g = divmod(c, 4)
        heads = [4 * g + i for i in range(HL)]
        # reorder W_qkv columns: [Q(h0..h3) | K(h0..h3) | V(h0..h3)]
        qcols = np.concatenate([W_qkv[:, h * 192:h * 192 + 64] for h in heads], 1)
        kcols = np.concatenate([W_qkv[:, h * 192 + 64:h * 192 + 128] for h in heads], 1)
        vcols = np.concatenate([W_qkv[:, h * 192 + 128:h * 192 + 192] for h in heads], 1)
        wsh = np.concatenate([qcols, kcols, vcols], 1)          # [1024, 768]
        bqv = np.concatenate([b_qkv[h * 192:h * 192 + 64] for h in heads])
        bkv = np.concatenate([b_qkv[h * 192 + 64:h * 192 + 128] for h in heads])
        bvv = np.concatenate([b_qkv[h * 192 + 128:h * 192 + 192] for h in heads])
        wo = W_out[g * 256:(g + 1) * 256, :]                    # [256, 1024]

        xT = x[b].T.reshape(8, 128, S).transpose(1, 0, 2)       # [128, 8, S]
        wsh3 = wsh.reshape(8, 128, 768).transpose(1, 0, 2)      # [128, 8, 768]
        wo3 = wo.reshape(2, 128, D).transpose(1, 0, 2)          # [128, 2, D]
        bq2 = np.concatenate([bqv, bkv]).reshape(4, 128).T      # [128, 4]
        bv2 = np.broadcast_to(bvv, (128, 256))                  # [128, 256]

        in_maps.append({
            "xT": np.ascontiguousarray(xT).astype(ml_dtypes.bfloat16),
            "w": np.ascontiguousarray(wsh3).astype(ml_dtypes.bfloat16),
            "wout": np.ascontiguousarray(wo3).astype(ml_dtypes.bfloat16),
            "bq": np.ascontiguousarray(bq2),
            "bv": np.ascontiguousarray(bv2),
            "cmask": np.ascontiguousarray(cmask),
        })

    _CACHE["in_maps"] = in_maps
    res = bass_utils.run_bass_kernel_spmd(nc, in_maps, core_ids=list(range(N_CORES)))

    out = np.zeros((B, S, D), np.float32)
    for c in range(N_CORES):
        b = c // 4
        oT = np.asarray(res.results[c]["outT"]).astype(np.float32)  # [128, 8, S]
        out[b] += oT.transpose(1, 0, 2).reshape(D, S).T
    out += b_out
    return out


# revision 86
# speedup vs baseline: 1.0257x; 1.0106x over previous
"""Multi-head causal attention (B=2, S=2048, D=1024, H=16) on 8 TRN2 NeuronCores.

Sharding: core c in 0..7 handles batch b = c // 4 and local head group
g = c % 4 (global heads 4g .. 4g+3).  Tensor-parallel over heads: each core
computes its heads' Q/K/V projections, causal attention, and a partial
output projection (W_out rows for its heads).  Host sums the 4 partials per
batch and adds b_out.

v2: bf16 storage everywhere (f32 PSUM accumulation), 128-granularity causal
trim, k-sliced startup DMA with k-outer chunk-0 projection, and phase
interleaving: qkv(c+1) / outproj(c-1) matmul blocks are emitted as filler
between attention ki-steps of chunk c so the tensor engine never waits on
the activation-engine exp cadence.
"""

from contextlib import ExitStack

import numpy as np
import ml_dtypes

import concourse.bass as bass
import concourse.mybir as mybir
import concourse.tile as tile
from concourse import bass_utils

F32 = mybir.dt.float32
BF16 = mybir.dt.bfloat16
EXP = mybir.ActivationFunctionType.Exp

B, S, D, H = 2, 2048, 1024, 16
HD = D // H          # 64
HL = 4               # heads per core
N_CORES = 8
SC = S // 512        # 4 q-chunks of 512
KT = S // 128        # 16 k-tiles of 128

_CACHE = {}

_NO_HOIST = {
    "AllEngineBarrier",
    "EventSemaphore",
    "UnconditionalBranch",
    "CompareAndBranch",
    "BranchHint",
    "IndirectBranch",
    "Halt",
    "Call",
    "OverlayCall",
    "NoOp",
}


def _fix_sync_waits(nc):
    """walrus codegen holds only one sync-wait per engine instruction; hoist
    excess waits onto same-engine NoOps inserted right before."""
    for fn in nc.m.functions:
        for blk in fn.blocks:
            insts = blk.instructions
            out = []
            changed = False
            for inst in insts:
                si = inst.sync_info
                if si is not None and inst.opcode not in _NO_HOIST:
                    waits = list(si.on_wait)
                    if len(waits) > 1:
                        for j, w in enumerate(waits[:-1]):
                            nop = mybir.InstNoOp(name=f"{inst.name}-wfix{j}")
                            nop.engine = inst.engine
                            nop.sync_info = mybir.SyncInfo(on_wait=[w], on_update=[])
                            out.append(nop)
                        inst.sync_info = mybir.SyncInfo(
                            on_wait=[waits[-1]], on_update=list(si.on_update)
                        )
                        changed = True
                out.append(inst)
            if changed:
                blk.instructions = out


def _build(reps=1, fix_waits=True, n_chunks=SC, trim=True, masks=True):
    nc = bass.Bass("TRN2", target_bir_lowering=False, debug=False,
                   num_devices=N_CORES)

    xT = nc.dram_tensor("xT", [128, 8, S], BF16, kind="ExternalInput").ap()
    w = nc.dram_tensor("w", [128, 8, 768], BF16, kind="ExternalInput").ap()
    wout = nc.dram_tensor("wout", [128, 2, D], BF16, kind="ExternalInput").ap()
    bq = nc.dram_tensor("bq", [128, 4], F32, kind="ExternalInput").ap()
    bv = nc.dram_tensor("bv", [128, 256], F32, kind="ExternalInput").ap()
    cmask = nc.dram_tensor("cmask", [128, 128], BF16, kind="ExternalInput").ap()
    outT = nc.dram_tensor("outT", [128, 8, S], BF16, kind="ExternalOutput").ap()

    with tile.TileContext(nc) as tc, ExitStack() as ctx:
        persist = ctx.enter_context(tc.tile_pool(name="persist", bufs=1))
        xpool = ctx.enter_context(tc.tile_pool(name="xp", bufs=3))
        epool = ctx.enter_context(tc.tile_pool(name="ep", bufs=3))
        rpool = ctx.enter_context(tc.tile_pool(name="rp", bufs=2))
        opool = ctx.enter_context(tc.tile_pool(name="op", bufs=4))
        # PSUM: scores 2x[128,1024] (4 banks) + AV accum 2x[128,512] (2) +
        # shared qkv/outproj/rb pool 2x[128,512] (2) = 8 banks
        ps_sc = ctx.enter_context(tc.tile_pool(name="ps_sc", bufs=2, space="PSUM"))
        ps_av = ctx.enter_context(tc.tile_pool(name="ps_av", bufs=2, space="PSUM"))
        ps_mm = ctx.enter_context(tc.tile_pool(name="ps_mm", bufs=2, space="PSUM"))

        w_sb = persist.tile([128, 8, 768], BF16, tag="w")
        wout_sb = persist.tile([128, 2, D], BF16, tag="wout")
        bq_sb = persist.tile([128, 4], F32, tag="bq")
        bv_sb = persist.tile([128, 256], F32, tag="bv")
        sel_sb = persist.tile([128, 256], BF16, tag="sel")
        cmask_sb = persist.tile([128, 128], BF16, tag="cmask")
        qT = persist.tile([128, 2, S], BF16, tag="qT")
        kT = persist.tile([128, 2, S], BF16, tag="kT")
        vn = persist.tile([128, KT, HL, 97], BF16, tag="vn")
        vraw = persist.tile([128, 2, S], BF16, tag="vraw")
        vnorm = persist.tile([128, 2, S], BF16, tag="vnorm")

        # device-built constants: the softmax-denominator ones column of the
        # augmented V — col 64 for even heads (den -> po row 64), col 96 for
        # odd heads (den -> po row 96, a valid mod-32 partition base) — and
        # the K=33 reciprocal-broadcast selector (rows 65..95 all zero).
        nc.vector.memset(sel_sb[64:96, 0:128], 0.0)
        nc.vector.memset(sel_sb[64:65, 0:64], 1.0)
        nc.vector.memset(sel_sb[96:97, 0:64], 0.0)
        nc.vector.memset(sel_sb[96:97, 64:128], 1.0)
        for h in range(HL):
            if h % 2 == 0:
                nc.vector.memset(vn[:, :, h, 64:65], 1.0)
            else:
                nc.vector.memset(vn[:, :, h, 64:96], 0.0)
                nc.vector.memset(vn[:, :, h, 96:97], 1.0)
        recrs = [persist.tile([128, 512], BF16, tag=f"recr{i}", name=f"recr{i}")
                 for i in range(2)]
        for r in recrs:
            nc.vector.memset(r[64:96, :], 0.0)

        # ---- startup DMA, k-sliced so the first matmuls start early ----
        xc0 = xpool.tile([128, 8, 512], BF16, tag="xc", name="xc0")
        nc.sync.dma_start(xc0[:, 0, :], xT[:, 0, 0:512])
        nc.scalar.dma_start(w_sb[:, 0, 0:512], w[:, 0, 0:512])
        nc.sync.dma_start(xc0[:, 1, :], xT[:, 1, 0:512])
        nc.scalar.dma_start(w_sb[:, 1, 0:512], w[:, 1, 0:512])
        for k2 in range(1, 4):
            ks2 = slice(2 * k2, 2 * k2 + 2)
            nc.sync.dma_start(xc0[:, ks2, :], xT[:, ks2, 0:512])
            nc.scalar.dma_start(w_sb[:, ks2, 0:512], w[:, ks2, 0:512])
        # V-projection columns trail the qk stream as small pair-slices so no
        # single bulk transfer can hold up the sequential v-blocks (each
        # v-block contracts over every k and so needs ALL of these)
        for k2 in range(4):
            ks2 = slice(2 * k2, 2 * k2 + 2)
            nc.scalar.dma_start(w_sb[:, ks2, 512:768], w[:, ks2, 512:768])
        nc.scalar.dma_start(bq_sb[:], bq)
        nc.scalar.dma_start(bv_sb[:], bv)
        nc.scalar.dma_start(cmask_sb[:], cmask)
        nc.scalar.dma_start(wout_sb[:], wout)

        # ---- PE clock warm-up: junk matmuls over the (memset) selector rows
        # while the first x/w slices stream in, so the p-state ramp completes
        # before the first real matmul (the result bank is never read)
        warm = ps_mm.tile([128, 512], F32, tag="mm", name="warm")
        for i in range(14):
            nc.tensor.matmul(warm[:, 0:256], sel_sb[64:96, 0:128],
                             sel_sb[64:96, 0:256], start=True, stop=True)

        # ---- chunk-0 qkv projection, k-outer (consumes slices as they land)
        # spA: [Q hp0 | K hp0], spB: [Q hp1 | K hp1]; pvA: [j0|j1], pvB: [j2|j3]
        # (hardware: at most ONE open matmul accumulation group per PSUM bank
        # — interleaved groups in a shared bank silently corrupt, so the four
        # qk accumulators get a bank each and V runs as sequential j-blocks)
        spA = ps_sc.tile([128, 1024], F32, tag="s", name="spA")
        spB = ps_sc.tile([128, 1024], F32, tag="s", name="spB")
        for k in range(8):
            st = k == 0
            sp_ = k == 7
            nc.tensor.matmul(spA[:, 0:512], w_sb[:, k, 0:128], xc0[:, k, :],
                             start=st, stop=sp_)
            nc.tensor.matmul(spA[:, 512:1024], w_sb[:, k, 256:384], xc0[:, k, :],
                             start=st, stop=sp_)
            nc.tensor.matmul(spB[:, 0:512], w_sb[:, k, 128:256], xc0[:, k, :],
                             start=st, stop=sp_)
            nc.tensor.matmul(spB[:, 512:1024], w_sb[:, k, 384:512], xc0[:, k, :],
                             start=st, stop=sp_)
        nc.vector.tensor_scalar_add(qT[:, 0, 0:512], spA[:, 0:512], bq_sb[:, 0:1])
        nc.vector.tensor_scalar_add(kT[:, 0, 0:512], spA[:, 512:1024], bq_sb[:, 2:3])
        nc.vector.tensor_scalar_add(qT[:, 1, 0:512], spB[:, 0:512], bq_sb[:, 1:2])
        nc.vector.tensor_scalar_add(kT[:, 1, 0:512], spB[:, 512:1024], bq_sb[:, 3:4])
        for st4 in range(4):
            pv = ps_av.tile([128, 512], F32, tag="av", name=f"pv0_{st4}")
            for k in range(8):
                nc.tensor.matmul(pv[:, 0:256],
                                 xc0[:, k, 128 * st4:128 * (st4 + 1)],
                                 w_sb[:, k, 512:768], start=(k == 0), stop=(k == 7))
            nc.vector.tensor_add(
                vn[:, st4, :, 0:64],
                pv[:, 0:256].rearrange("p (h d) -> p h d", h=4),
                bv_sb[:].rearrange("p (h d) -> p h d", h=4))

        xcs = {0: xc0}

        # ---- filler blocks: qkv projection of a later chunk / output
        # projection of an earlier chunk, emitted between attention steps ----
        def qk_halves(cn, m):
            # m: 0=Q hp0, 1=Q hp1, 2=K hp0, 3=K hp1 (matches w col + bias col)
            # split into two 4-k-step closures for finer filler granularity
            hold = {}

            def emit_a():
                hold["pm"] = ps_mm.tile([128, 512], F32, tag="mm",
                                        name=f"qk{cn}_{m}")
                for k in range(4):
                    nc.tensor.matmul(hold["pm"][:], w_sb[:, k, 128 * m:128 * (m + 1)],
                                     xcs[cn][:, k, :], start=(k == 0), stop=False)

            def emit_b():
                qs = slice(cn * 512, (cn + 1) * 512)
                pm = hold["pm"]
                for k in range(4, 8):
                    nc.tensor.matmul(pm[:], w_sb[:, k, 128 * m:128 * (m + 1)],
                                     xcs[cn][:, k, :], start=False, stop=(k == 7))
                dst = qT[:, m, qs] if m < 2 else kT[:, m - 2, qs]
                nc.vector.tensor_scalar_add(dst, pm[:], bq_sb[:, m:m + 1])
            return [emit_a, emit_b]

        def v_block(cn, j):
            def emit():
                pv = ps_mm.tile([128, 512], F32, tag="mm", name=f"v{cn}_{j}")
                for k in range(8):
                    nc.tensor.matmul(pv[:, 0:256],
                                     xcs[cn][:, k, 128 * j:128 * (j + 1)],
                                     w_sb[:, k, 512:768], start=(k == 0), stop=(k == 7))
                st4 = 4 * cn + j
                nc.vector.tensor_add(
                    vn[:, st4, :, 0:64],
                    pv[:, 0:256].rearrange("p (h d) -> p h d", h=4),
                    bv_sb[:].rearrange("p (h d) -> p h d", h=4))
            return emit

        ou_hold = {}
        pu_hold = {}

        def outproj_block(cn, m, copy_eng="dve", psum="mm"):
            # even m allocates a 2-block staging tile; odd m completes it and
            # issues one paired DMA (halves the per-transfer HWDGE overhead).
            # psum="sc": after the last scores, the 4 score banks are free —
            # pair two blocks per [128,1024] tile for deeper PU buffering.
            def emit():
                qs = slice(cn * 512, (cn + 1) * 512)
                if psum == "mm":
                    pu = ps_mm.tile([128, 512], F32, tag="mm",
                                    name=f"pu{cn}_{m}")[:]
                elif psum == "alt":
                    # tail: the AV banks are free — alternate pools so four
                    # blocks can be in flight instead of two
                    pool_ = ps_mm if m % 2 == 0 else ps_av
                    pu = pool_.tile([128, 512], F32, tag=("mm" if m % 2 == 0 else "av"),
                                    name=f"pu{cn}_{m}")[:]
                else:
                    if m % 2 == 0:
                        pu_hold[cn] = ps_sc.tile([128, 1024], F32, tag="s",
                                                 name=f"pu2{cn}_{m}")
                    pu = pu_hold[cn][:, 512 * (m % 2):512 * (m % 2) + 512]
                for t in range(2):
                    nc.tensor.matmul(pu, wout_sb[:, t, 128 * m:128 * (m + 1)],
                                     vnorm[:, t, qs], start=(t == 0), stop=(t == 1))
                if m % 2 == 0:
                    ou_hold[cn] = opool.tile([128, 2, 512], BF16, tag="ou",
                                             name=f"ou{cn}_{m}")
                ou = ou_hold[cn]
                dst = ou[:, m % 2, :]
                if copy_eng == "dve":
                    nc.vector.tensor_copy(dst, pu)
                elif copy_eng == "act":
                    nc.scalar.copy(dst, pu)
                if m % 2 == 1:
                    nc.sync.dma_start(outT[:, m - 1:m + 1, qs], ou[:])
            return emit

        # pending normalization chain of the previous head pair — emitted
        # right AFTER the next head pair's first exp is queued, so the Act
        # engine starts the next exp before the drain copies, and the PE has
        # scores/filler work while the reciprocal chain flows (crosses chunk
        # boundaries too)
        pending = {"norm": None}

        # ---- attention for one q-chunk, with filler drained between steps
        def attn_qc(qc, fillers, reserve=()):
            n_ki = 4 * qc + 4
            nsteps = 2 * n_ki + 2
            state = {"step": 0, "drained": 0}

            def tick(n=None):
                state["step"] += 1
                if n is None:
                    # at least one filler by step 1: right after a head-pair
                    # boundary the PE otherwise idles on the exp/recip chains
                    target = max(len(fillers) * state["step"] // nsteps,
                                 min(2, state["step"]))
                else:
                    target = state["drained"] + n
                while state["drained"] < min(target, len(fillers)):
                    fillers[state["drained"]]()
                    state["drained"] += 1

            qs = slice(qc * 512, (qc + 1) * 512)
            for hp in range(2):
                # po tiles are allocated lazily at the first AV so the pool
                # WAR lands after the previous pair's (deferred) drain copies
                po = [None, None]
                recr = recrs[(2 * qc + hp) % 2]
                es_hold = [None] * n_ki

                def do_av(ki, qc=qc, hp=hp, po=po, n_ki=n_ki, es_hold=es_hold):
                    if po[0] is None:
                        po[0] = ps_av.tile([128, 512], F32, tag="av",
                                           name=f"po{qc}{hp}0")
                        po[1] = ps_av.tile([128, 512], F32, tag="av",
                                           name=f"po{qc}{hp}1")
                    e, o, wdt = es_hold[ki]
                    for i in range(2):
                        h = 2 * hp + i
                        # V_aug.T @ E: rows 0..63 values, row 64 (even) or 96
                        # (odd) the softmax denominator (ones column of V_aug)
                        nc.tensor.matmul(
                            po[i][0:65 + 32 * i, o:512], vn[:, ki, h, 0:65 + 32 * i],
                            e[:, i * 512:i * 512 + wdt],
                            start=(ki == 0), stop=(ki == n_ki - 1),
                            skip_group_check=True)

                for ki in range(n_ki):
                    j = ki - 4 * qc
                    o = 128 * j if (trim and j >= 0) else 0
                    wdt = 512 - o
                    ks = slice(ki * 128, (ki + 1) * 128)
                    qsub = slice(qc * 512 + o, (qc + 1) * 512)
                    # head slabs live at bank-aligned offsets i*512 — the two
                    # tile_position score groups must not share a PSUM bank
                    sp = ps_sc.tile([128, 1024], F32, tag="s",
                                    name=f"sp{qc}{hp}{ki}")
                    for i in range(2):   # head within pair (row-packed)
                        vp = 64 * i
                        nc.tensor.matmul(
                            sp[:, i * 512:i * 512 + wdt],
                            kT[vp:vp + 64, hp, ks], qT[vp:vp + 64, hp, qsub],
                            start=True, stop=True, tile_position=(vp, 0))
                    e = epool.tile([128, 1024], BF16, tag="e",
                                   name=f"e{qc}{hp}{ki}")
                    if wdt == 512:
                        nc.scalar.activation(e[:], sp[:], EXP, scale=0.125)
                    else:
                        sp3 = sp[:].rearrange("p (t q) -> p t q", t=2)
                        e3 = e[:].rearrange("p (t q) -> p t q", t=2)
                        nc.scalar.activation(e3[:, :, 0:wdt], sp3[:, :, 0:wdt],
                                             EXP, scale=0.125)
                    if masks and j >= 0:
                        # diagonal tile: with o=128j the invalid region is
                        # always the leading 128-col triangle (kk > qq)
                        mw = min(128, wdt)
                        for i in range(2):
                            es = e[:, i * 512:i * 512 + mw]
                            nc.vector.tensor_mul(es, es, cmask_sb[:, 0:mw])
                    es_hold[ki] = (e, o, wdt)
                    if ki == 0 and pending["norm"] is not None:
                        pending["norm"]()
                        pending["norm"] = None
                    if ki >= 1:
                        do_av(ki - 1)
                    tick()
                do_av(n_ki - 1)
                tick()

                # normalization: per-head reciprocal of the denominator rows,
                # broadcast onto value partitions via one K=33 selector matmul.
                # Denominators: even head on po[0] row 64, odd head on po[1]
                # row 96 (the odd V_aug ones column sits at col 96), so both
                # reciprocals stay partition-aligned (rows 65..95 are zero).
                def norm(qc=qc, hp=hp, po=po, recr=recr, qs=qs,
                         res=tuple(reserve) if (hp == 1 and reserve) else ()):
                    stage = rpool.tile([128, 512], BF16, tag="stage",
                                       name=f"st{qc}{hp}")
                    nc.scalar.copy(stage[0:64, :], po[1][0:64, :])
                    # sync ring: out-DMAs queued behind this were emitted in
                    # the same window, so the short stage-copy wait cannot
                    # head-of-line-block them for long
                    nc.sync.dma_start(vraw[64:128, hp, qs], stage[0:64, :])
                    with nc.allow_low_precision(reason="bf16 softmax recip"):
                        nc.vector.reciprocal(recr[96:97, :], po[1][96:97, :])
                        nc.vector.reciprocal(recr[64:65, :], po[0][64:65, :])
                    nc.scalar.copy(vraw[0:64, hp, qs], po[0][0:64, :])
                    if res:
                        res[0]()
                        res[1]()
                    else:
                        tick(n=2)
                    rb = ps_av.tile([128, 512], F32, tag="av", name=f"rb{qc}{hp}")
                    nc.tensor.matmul(rb[:], sel_sb[64:97, 0:128], recr[64:97, :],
                                     start=True, stop=True, tile_position=(64, 0))
                    for r in res[2:]:
                        r()
                    nc.vector.tensor_mul(vnorm[:, hp, qs], vraw[:, hp, qs], rb[:])
                pending["norm"] = norm
            # any fillers not yet drained
            while state["drained"] < len(fillers):
                fillers[state["drained"]]()
                state["drained"] += 1

        # ---- main schedule: attn(c) with qkv(c+1) as filler; all deferrable
        # output projections (chunks 0..2) land in attn(3), whose exp cadence
        # otherwise starves the tensor engine; outproj(3) is the tail.  In
        # attn(3) the copies ride the idle Pool engine so the DVE recip ->
        # vnorm chain stays short; the last few blocks are reserved to keep
        # the PE warm through the final normalization chain.
        for c in range(n_chunks):
            fillers, reserve = [], []
            if c + 1 < n_chunks:
                xc = xpool.tile([128, 8, 512], BF16, tag="xc", name=f"xc{c+1}")
                qsn = slice((c + 1) * 512, (c + 2) * 512)
                nc.sync.dma_start(xc[:, 0:4, :], xT[:, 0:4, qsn])
                nc.sync.dma_start(xc[:, 4:8, :], xT[:, 4:8, qsn])
                xcs[c + 1] = xc
                for m in range(4):
                    fillers += qk_halves(c + 1, m)
                fillers += [v_block(c + 1, j) for j in range(4)]
            else:
                # GPSIMD cannot touch PSUM, so drain copies ride DVE (fillers)
                # and Act (reserve, where the exp pipeline has already drained)
                for cn in range(n_chunks - 2):
                    fillers += [outproj_block(cn, m, "dve") for m in range(8)]
                fillers += [outproj_block(n_chunks - 2, m, "dve") for m in range(4)]
                reserve = [outproj_block(n_chunks - 2, m, "act") for m in range(4, 8)]
            attn_qc(c, fillers, reserve)
        # the last head pair's normalization, then the tail output projection
        pending["norm"]()
        pending["norm"] = None
        for m in range(8):
            outproj_block(n_chunks - 1, m, copy_eng=("dve" if m % 2 else "act"),
                          psum="alt")()

    if fix_waits:
        _fix_sync_waits(nc)
    return nc


def _get_nc():
    if "nc" not in _CACHE:
        _CACHE["nc"] = _build()
    return _CACHE["nc"]


def _make_cmask() -> np.ndarray:
    """cmask[128, 128]: c[kk, qq] = 1 iff kk <= qq (relative causal triangle
    applied to the leading 128 cols of every diagonal score tile)."""
    kk = np.arange(128)[:, None]
    qq = np.arange(128)[None, :]
    return (kk <= qq).astype(np.float32)


def kernel(x, W_qkv, b_qkv, W_out, b_out):
    x = np.asarray(x, np.float32)
    W_qkv = np.asarray(W_qkv, np.float32)
    b_qkv = np.asarray(b_qkv, np.float32)
    W_out = np.asarray(W_out, np.float32)
    b_out = np.asarray(b_out, np.float32)

    nc = _get_nc()
    cmask = _make_cmask().astype(ml_dtypes.bfloat16)

    in_maps = []
    for c in range(N_CORES):
        b, g = divmod(c, 4)
        heads = [4 * g + i for i in range(HL)]
        # reorder W_qkv columns: [Q(h0..h3) | K(h0..h3) | V(h0..h3)]
        qcols = np.concatenate([W_qkv[:, h * 192:h * 192 + 64] for h in heads], 1)
        kcols = np.concatenate([W_qkv[:, h * 192 + 64:h * 192 + 128] for h in heads], 1)
        vcols = np.concatenate([W_qkv[:, h * 192 + 128:h * 192 + 192] for h in heads], 1)
        wsh = np.concatenate([qcols, kcols, vcols], 1)          # [1024, 768]
        bqv = np.concatenate([b_qkv[h * 192:h * 192 + 64] for h in heads])
        bkv = np.concatenate([b_qkv[h * 192 + 64:h * 192 + 128] for h in heads])
        bvv = np.concatenate([b_qkv[h * 192 + 128:h * 192 + 192] for h in heads])
        wo = W_out[g * 256:(g + 1) * 256, :]                    # [256, 1024]

        xT = x[b].T.reshape(8, 128, S).transpose(1, 0, 2)       # [128, 8, S]
        wsh3 = wsh.reshape(8, 128, 768).transpose(1, 0, 2)      # [128, 8, 768]
        wo3 = wo.reshape(2, 128, D).transpose(1, 0, 2)          # [128, 2, D]
        bq2 = np.concatenate([bqv, bkv]).reshape(4, 128).T      # [128, 4]
        bv2 = np.broadcast_to(bvv, (128, 256))                  # [128, 256]

        in_maps.append({
            "xT": np.ascontiguousarray(xT).astype(ml_dtypes.bfloat16),
            "w": np.ascontiguousarray(wsh3).astype(ml_dtypes.bfloat16),
            "wout": np.ascontiguousarray(wo3).astype(ml_dtypes.bfloat16),
            "bq": np.ascontiguousarray(bq2),
            "bv": np.ascontiguousarray(bv2),
            "cmask": np.ascontiguousarray(cmask),
        })

    _CACHE["in_maps"] = in_maps
    res = bass_utils.run_bass_kernel_spmd(nc, in_maps, core_ids=list(range(N_CORES)))

    out = np.zeros((B, S, D), np.float32)
    for c in range(N_CORES):
        b = c // 4
        oT = np.asarray(res.results[c]["outT"]).astype(np.float32)  # [128, 8, S]
        out[b] += oT.transpose(1, 0, 2).reshape(D, S).T
    out += b_out
    return out
